# revision 16
# baseline (speedup 1.0000x reference)
"""Self-contained Trainium2 Bass kernel for nn_ANEDecoderLayer (ANE decoder layer).

Shapes (hardcoded): B=2, C=2048, S=1024, H=16, HD=128, FF=8192, fp32 I/O.

Sharding: hybrid batch(2) x tensor-parallel(4) over 8 NeuronCores.
  core = g*4 + r:  g = batch index, r = TP rank.
  Within each group of 4 cores: heads sharded 4/core, d_ff sharded 2048/core.
  Block outputs (row-split Wo / W_down partial sums) are AllReduced in bf16
  within each 4-core group: replica_groups [[0,1,2,3],[4,5,6,7]].

Host-side preprocessing:
  - RMSNorm weights folded into the following matmul weights.
  - Weights pre-transposed, pre-tiled for lhsT layout, cast to bf16.
  - sin_k/cos_k pre-scaled by 1/sqrt(HD) (folds attention scale into K).
  - kv cache scatter (kv_write_idx) folded into a row-permutation of the mask.
  - additive masks converted to multiplicative exp(mask) tile patterns with
    per-tile classification (all-ones -> no op, all-zero -> tile skipped,
    else multiply by a deduplicated pattern tile).

Compute: matmuls in bf16 (fp32 PSUM accumulation), RMSNorm statistics in
fp32, softmax without max-subtraction (scores are O(5); exp(-1e9)=0 handled
by tile skipping), residual stream held in fp32 in SBUF.
"""

import numpy as np
import ml_dtypes

import concourse.mybir as mybir
import concourse.tile as tile
from concourse import bacc
from concourse.bass_utils import run_bass_kernel_spmd

# ---------------------------------------------------------------- constants
B, C, S, H, HD, FF = 2, 2048, 1024, 16, 128, 8192
EPS = 1e-5
SCALE = 1.0 / float(np.sqrt(HD))

NCORES = 8
TPG = 4                      # tensor-parallel group size
HPC = H // TPG               # heads per core = 4
OCA = HPC * HD               # attention out-channels per core = 512
FFC = FF // TPG              # ff channels per core = 2048

CT = C // 128                # 16 c-chunks
ST = S // 512                # 2 s-chunks of 512
KT = S // 128                # 8 k-chunks
FFT = FFC // 128             # 16 ff-chunks per core

F32 = mybir.dt.float32
BF = mybir.dt.bfloat16
AF = mybir.ActivationFunctionType
MULT = mybir.AluOpType.mult
ADD = mybir.AluOpType.add
BF_NP = ml_dtypes.bfloat16

REPLICA_GROUPS = [[0, 1, 2, 3], [4, 5, 6, 7]]

_CACHE: dict = {}


# ---------------------------------------------------------------- host prep
def _pack_lhsT(wT: np.ndarray) -> np.ndarray:
    """wT: (K, M) contraction-major weight. Returns (M//128, 128, K) bf16 where
    pack[m][p, kc*128+f] = wT[kc*128+p, m*128+f]; a DMA of pack[m] gives an
    SBUF tile whose slice [:, kc*128:(kc+1)*128] is the lhsT for contraction
    chunk kc -> output chunk m."""
    K, M = wT.shape
    Kt, Mt = K // 128, M // 128
    t = wT.reshape(Kt, 128, Mt, 128)              # [kc, p, m, f]
    t = t.transpose(2, 1, 0, 3).reshape(Mt, 128, K)
    return np.ascontiguousarray(t.astype(BF_NP))


def _classify_mask(mask_eff: np.ndarray):
    """mask_eff: (S, S) additive mask, (k, q) orientation. Returns
    (cls, patterns): cls[qc][kc] in {'c' (clean), 's' (skip), int idx};
    patterns: (NB, 128, 512) bf16 multiplicative tiles."""
    mm = np.exp(np.minimum(mask_eff.astype(np.float64), 0.0)).astype(np.float32)
    # positive masks would overflow exp; reference masks are <= 0
    if mask_eff.max() > 0:
        mm = np.exp(mask_eff.astype(np.float64)).astype(np.float32)
    patterns = []
    keys = {}
    cls = [[None] * KT for _ in range(ST)]
    for qc in range(ST):
        for kc in range(KT):
            sub = mm[kc * 128:(kc + 1) * 128, qc * 512:(qc + 1) * 512]
            if np.all(sub == 1.0):
                cls[qc][kc] = 'c'
            elif np.all(sub == 0.0):
                cls[qc][kc] = 's'
            else:
                kb = sub.tobytes()
                if kb not in keys:
                    keys[kb] = len(patterns)
                    patterns.append(sub.astype(BF_NP))
                cls[qc][kc] = keys[kb]
    if patterns:
        pat = np.stack(patterns)
    else:
        pat = np.zeros((1, 128, 512), BF_NP)
    return cls, pat


def _prep_host(inputs):
    """Returns (shared_map, per_rank_maps, sa_cls, ca_cls)."""
    g = lambda k: np.asarray(inputs[k], dtype=np.float32)

    sinq = np.ascontiguousarray(g('sin_q').reshape(HD, S))
    cosq = np.ascontiguousarray(g('cos_q').reshape(HD, S))
    sink = np.ascontiguousarray(g('sin_k').reshape(HD, S) * SCALE)
    cosk = np.ascontiguousarray(g('cos_k').reshape(HD, S) * SCALE)

    idx = np.asarray(inputs['kv_write_idx']).astype(np.int64)
    if not np.array_equal(np.sort(idx), np.arange(S)):
        raise NotImplementedError("kv_write_idx must be a permutation of arange(S)")
    sa_mask = g('self_attn_mask').reshape(S, S)[idx, :]     # effective (k, q) mask
    ca_mask = g('cross_attn_mask').reshape(S, S)
    sa_cls, sa_pat = _classify_mask(sa_mask)
    ca_cls, ca_pat = _classify_mask(ca_mask)

    P_rot = np.zeros((HD, HD), np.float32)
    P_rot[np.arange(64), np.arange(64, 128)] = -1.0
    P_rot[np.arange(64, 128), np.arange(64)] = 1.0

    shared = {
        'sinq': sinq.astype(BF_NP), 'cosq': cosq.astype(BF_NP),
        'sink': sink.astype(BF_NP), 'cosk': cosk.astype(BF_NP),
        'ones_col': np.ones((128, 1), np.float32),
        'ones_col_bf': np.ones((128, 1), BF_NP),
        'ones_row': np.ones((1, 128), np.float32),
        'ident': np.eye(128).astype(BF_NP),
        'protT': np.ascontiguousarray(P_rot.T).astype(BF_NP),
        'mask_sa': sa_pat, 'mask_ca': ca_pat,
    }

    w_sa, w_ca, w_mlp = g('w_sa'), g('w_ca'), g('w_mlp')
    per_rank = []
    for r in range(TPG):
        asl = slice(r * OCA, (r + 1) * OCA)
        fsl = slice(r * FFC, (r + 1) * FFC)
        m = {}
        for tag, wn, on in (('sa', 'w_sa', None), ('ca', 'w_ca', None)):
            wnorm = w_sa if tag == 'sa' else w_ca
            for p in ('q', 'k', 'v'):
                W = g(f'w{p}_{tag}')[asl, :] * wnorm[None, :]
                m[f'w{p}_{tag}'] = _pack_lhsT(np.ascontiguousarray(W.T))
            Wo = g(f'wo_{tag}')[:, asl]
            m[f'wo_{tag}'] = _pack_lhsT(np.ascontiguousarray(Wo.T))
        for p, key in (('g', 'w_gate'), ('u', 'w_up')):
            W = g(key)[fsl, :] * w_mlp[None, :]
            m[f'w{p}'] = _pack_lhsT(np.ascontiguousarray(W.T))
        Wd = g('w_down')[:, fsl]
        m['wd'] = _pack_lhsT(np.ascontiguousarray(Wd.T))
        per_rank.append(m)

    return shared, per_rank, sa_cls, ca_cls


# ---------------------------------------------------------------- builder
def _build(sa_cls, ca_cls, nb_sa, nb_ca):
    nc = bacc.Bacc("TRN2", target_bir_lowering=False, debug=False,
                   num_devices=NCORES)

    d_x = nc.declare_dram_parameter("x", [C, S], F32, isOutput=False)
    d_tab = {k: nc.declare_dram_parameter(k, [HD, S], BF, isOutput=False)
             for k in ('sinq', 'cosq', 'sink', 'cosk')}
    d_oc = nc.declare_dram_parameter("ones_col", [128, 1], F32, isOutput=False)
    d_ocb = nc.declare_dram_parameter("ones_col_bf", [128, 1], BF, isOutput=False)
    d_or = nc.declare_dram_parameter("ones_row", [1, 128], F32, isOutput=False)
    d_id = nc.declare_dram_parameter("ident", [128, 128], BF, isOutput=False)
    d_pr = nc.declare_dram_parameter("protT", [128, 128], BF, isOutput=False)
    d_msa = nc.declare_dram_parameter("mask_sa", [nb_sa, 128, 512], BF, isOutput=False)
    d_mca = nc.declare_dram_parameter("mask_ca", [nb_ca, 128, 512], BF, isOutput=False)
    d_w = {}
    for t in ('sa', 'ca'):
        for p in ('q', 'k', 'v'):
            d_w[f'w{p}_{t}'] = nc.declare_dram_parameter(
                f'w{p}_{t}', [OCA // 128, 128, C], BF, isOutput=False)
        d_w[f'wo_{t}'] = nc.declare_dram_parameter(
            f'wo_{t}', [CT, 128, OCA], BF, isOutput=False)
    for k in ('wg', 'wu', 'wd'):
        kdim = C if k != 'wd' else FFC
        d_w[k] = nc.declare_dram_parameter(k, [FFT, 128, kdim], BF, isOutput=False)
    d_out = nc.declare_dram_parameter("out", [C, S], F32, isOutput=True)

    with tile.TileContext(nc) as tc:
        with (
            tc.tile_pool(name="const", bufs=1) as cpool,
            tc.tile_pool(name="xp", bufs=1) as xpool,
            tc.tile_pool(name="hp", bufs=1) as hpool,
            tc.tile_pool(name="wb", bufs=3) as wpool,
            tc.tile_pool(name="oo", bufs=2) as opool,
            tc.tile_pool(name="sm", bufs=2) as spool,
            tc.tile_pool(name="dram", bufs=1, space="DRAM") as dpool,
            tc.tile_pool(name="psA", bufs=5, space="PSUM") as psA,
            tc.tile_pool(name="psS", bufs=1, space="PSUM") as psS,
            tc.tile_pool(name="psT", bufs=2, space="PSUM") as psT,
        ):
            # ---------------- constants / tables ----------------
            def ptile(pool, shape, dt, name):
                return pool.tile(shape, dt, name=name, tag=name)

            ones_col = ptile(cpool, [128, 1], F32, "ones_col")
            ones_col_bf = ptile(cpool, [128, 1], BF, "ones_col_bf")
            ones_row = ptile(cpool, [1, 128], F32, "ones_row")
            ident = ptile(cpool, [128, 128], BF, "ident")
            protT = ptile(cpool, [128, 128], BF, "protT")
            eps_t = ptile(cpool, [128, 1], F32, "eps_t")
            nc.sync.dma_start(ones_col[:], d_oc.ap())
            nc.sync.dma_start(ones_col_bf[:], d_ocb.ap())
            nc.sync.dma_start(ones_row[:], d_or.ap())
            nc.sync.dma_start(ident[:], d_id.ap())
            nc.sync.dma_start(protT[:], d_pr.ap())
            nc.vector.memset(eps_t[:], EPS)
            tabs = {}
            for k in d_tab:
                tabs[k] = ptile(cpool, [HD, S], BF, f"tab_{k}")
                nc.sync.dma_start(tabs[k][:], d_tab[k].ap())
            used_sa = {c for row in sa_cls for c in row if isinstance(c, int)}
            used_ca = {c for row in ca_cls for c in row if isinstance(c, int)}
            msk_sa, msk_ca = {}, {}
            for i in sorted(used_sa):
                msk_sa[i] = ptile(cpool, [128, 512], BF, f"msa{i}")
                nc.sync.dma_start(msk_sa[i][:], d_msa.ap()[i])
            for i in sorted(used_ca):
                msk_ca[i] = ptile(cpool, [128, 512], BF, f"mca{i}")
                nc.sync.dma_start(msk_ca[i][:], d_mca.ap()[i])

            # ---------------- residual stream x ----------------
            xt = [ptile(xpool, [128, S], F32, f"x{cc}") for cc in range(CT)]
            for cc in range(CT):
                nc.sync.dma_start(xt[cc][:], d_x.ap()[cc * 128:(cc + 1) * 128, :])
            ht = [ptile(hpool, [128, S], BF, f"h{cc}") for cc in range(CT)]

            # ---------------- helpers ----------------
            def rmsnorm(scope):
                """ht[:] = xt * rsqrt(mean_c(xt^2) + eps), bf16."""
                with nc.named_scope(scope):
                    for sc in range(ST):
                        ss = psS.tile([1, 512], F32, tag="sum")
                        for cc in range(CT):
                            sq = spool.tile([128, 512], F32, tag="sq")
                            nc.scalar.activation(
                                sq[:], xt[cc][:, sc * 512:(sc + 1) * 512], AF.Square)
                            nc.tensor.matmul(ss[:], ones_col[:], sq[:],
                                             start=(cc == 0), stop=(cc == CT - 1))
                        rs = spool.tile([1, 512], F32, tag="rs")
                        nc.scalar.activation(rs[:], ss[:], AF.Sqrt,
                                             bias=eps_t[:1, :], scale=1.0 / C)
                        rr = spool.tile([1, 512], F32, tag="rr")
                        nc.vector.reciprocal(rr[:], rs[:])
                        bc = psA.tile([128, 512], F32, tag="acc")
                        nc.tensor.matmul(bc[:], ones_row[:], rr[:],
                                         start=True, stop=True)
                        for cc in range(CT):
                            nc.vector.tensor_tensor(
                                ht[cc][:, sc * 512:(sc + 1) * 512],
                                xt[cc][:, sc * 512:(sc + 1) * 512],
                                bc[:], op=MULT)

            def residual_add(b_out, scope, final=False):
                """xt += AR result (bf16 in dram b_out); if final, write to out."""
                with nc.named_scope(scope):
                    for cc in range(CT):
                        ar = opool.tile([128, S], BF, tag="ar")
                        nc.sync.dma_start(ar[:], b_out[cc * 128:(cc + 1) * 128, :])
                        if final:
                            ot = opool.tile([128, S], F32, tag="obuf")
                            nc.vector.tensor_tensor(ot[:], xt[cc][:], ar[:], op=ADD)
                            nc.sync.dma_start(
                                d_out.ap()[cc * 128:(cc + 1) * 128, :], ot[:])
                        else:
                            nc.vector.tensor_tensor(xt[cc][:], xt[cc][:], ar[:], op=ADD)

            def attention(t, cls, msk, apool):
                """One attention block (t='sa'|'ca'). Returns bounce-out dram tile."""
                rmsnorm(f"{t}_norm")
                qk_rope = {}
                vT = [apool.tile([128, S], BF, name=f"vT{t}{oc}", tag=f"vT{oc}",
                                 bufs=1) for oc in range(HPC)]
                with nc.named_scope(f"{t}_qkv"):
                    for p in ('q', 'k', 'v'):
                        for oc in range(HPC):
                            if p != 'v':
                                dst = apool.tile([128, S], BF,
                                                 name=f"{p}r{t}{oc}",
                                                 tag=f"{p}r{oc}", bufs=1)
                                qk_rope[(p, oc)] = dst
                            wsb = wpool.tile([128, C], BF, tag="wbig")
                            nc.sync.dma_start(wsb[:], d_w[f'w{p}_{t}'].ap()[oc])
                            for sc in range(ST):
                                s0 = slice(sc * 512, (sc + 1) * 512)
                                ps = psA.tile([128, 512], F32, tag="acc")
                                for cc in range(CT):
                                    nc.tensor.matmul(
                                        ps[:], wsb[:, cc * 128:(cc + 1) * 128],
                                        ht[cc][:, s0],
                                        start=(cc == 0), stop=(cc == CT - 1))
                                if p == 'v':
                                    vsb = spool.tile([128, 512], BF, tag="lin")
                                    nc.scalar.activation(vsb[:], ps[:], AF.Copy)
                                    for j in range(4):
                                        kc = sc * 4 + j
                                        tp = psT.tile([128, 128], BF, tag="tr")
                                        nc.tensor.transpose(
                                            tp[:], vsb[:, j * 128:(j + 1) * 128],
                                            ident[:])
                                        nc.scalar.activation(
                                            vT[oc][:, kc * 128:(kc + 1) * 128],
                                            tp[:], AF.Copy)
                                else:
                                    lin = spool.tile([128, 512], BF, tag="lin")
                                    nc.scalar.activation(lin[:], ps[:], AF.Copy)
                                    rot = psA.tile([128, 512], F32, tag="acc")
                                    nc.tensor.matmul(rot[:], protT[:], lin[:],
                                                     start=True, stop=True)
                                    sin = tabs['sinq' if p == 'q' else 'sink']
                                    cos = tabs['cosq' if p == 'q' else 'cosk']
                                    dst = qk_rope[(p, oc)]
                                    nc.vector.tensor_tensor(
                                        dst[:, s0], lin[:], cos[:, s0], op=MULT)
                                    s2 = spool.tile([128, 512], BF, tag="rsc")
                                    nc.vector.tensor_tensor(
                                        s2[:], rot[:], sin[:, s0], op=MULT)
                                    nc.vector.tensor_tensor(
                                        dst[:, s0], dst[:, s0], s2[:], op=ADD)
                att = [apool.tile([128, S], BF, name=f"att{t}{oc}",
                                  tag=f"att{oc}", bufs=1) for oc in range(HPC)]
                with nc.named_scope(f"{t}_attn"):
                    for oc in range(HPC):
                        qr, kr = qk_rope[('q', oc)], qk_rope[('k', oc)]
                        for qc in range(ST):
                            s0 = slice(qc * 512, (qc + 1) * 512)
                            valid = [kc for kc in range(KT) if cls[qc][kc] != 's']
                            probs = {}
                            for kc in valid:
                                sp = psA.tile([128, 512], F32, tag="acc")
                                nc.tensor.matmul(
                                    sp[:], kr[:, kc * 128:(kc + 1) * 128],
                                    qr[:, s0], start=True, stop=True)
                                pt = apool.tile([128, 512], BF, tag="probs",
                                                bufs=8)
                                nc.scalar.activation(pt[:], sp[:], AF.Exp)
                                if cls[qc][kc] != 'c':
                                    nc.vector.tensor_tensor(
                                        pt[:], pt[:], msk[cls[qc][kc]][:], op=MULT)
                                probs[kc] = pt
                            dn = psS.tile([1, 512], F32, tag="sum")
                            for i, kc in enumerate(valid):
                                nc.tensor.matmul(dn[:], ones_col_bf[:],
                                                 probs[kc][:],
                                                 start=(i == 0),
                                                 stop=(i == len(valid) - 1))
                            ra = spool.tile([1, 512], F32, tag="ra")
                            nc.vector.reciprocal(ra[:], dn[:])
                            rb = psA.tile([128, 512], F32, tag="acc")
                            nc.tensor.matmul(rb[:], ones_row[:], ra[:],
                                             start=True, stop=True)
                            # DVE reads only one PSUM input; stage bcast in SBUF
                            rbs = spool.tile([128, 512], F32, tag="rbs")
                            nc.scalar.activation(rbs[:], rb[:], AF.Copy)
                            pa = psA.tile([128, 512], F32, tag="acc")
                            for i, kc in enumerate(valid):
                                nc.tensor.matmul(
                                    pa[:], vT[oc][:, kc * 128:(kc + 1) * 128],
                                    probs[kc][:],
                                    start=(i == 0), stop=(i == len(valid) - 1))
                            nc.vector.tensor_tensor(att[oc][:, s0], pa[:], rbs[:],
                                                    op=MULT)
                b_in = dpool.tile([C, S], BF, name=f"bin_{t}", tag=f"bin_{t}")
                with nc.named_scope(f"{t}_wo"):
                    for cc in range(CT):
                        wsb = wpool.tile([128, OCA], BF, tag="wsm")
                        nc.sync.dma_start(wsb[:], d_w[f'wo_{t}'].ap()[cc])
                        osb = opool.tile([128, S], BF, tag="obuf")
                        for sc in range(ST):
                            s0 = slice(sc * 512, (sc + 1) * 512)
                            ps = psA.tile([128, 512], F32, tag="acc")
                            for ac in range(HPC):
                                nc.tensor.matmul(
                                    ps[:], wsb[:, ac * 128:(ac + 1) * 128],
                                    att[ac][:, s0],
                                    start=(ac == 0), stop=(ac == HPC - 1))
                            nc.scalar.activation(osb[:, s0], ps[:], AF.Copy)
                        nc.sync.dma_start(b_in[cc * 128:(cc + 1) * 128, :], osb[:])
                b_out = dpool.tile([C, S], BF, name=f"bout_{t}", tag=f"bout_{t}")
                nc.gpsimd.collective_compute(
                    "AllReduce", ADD, replica_groups=REPLICA_GROUPS,
                    ins=[b_in[:].opt()], outs=[b_out[:].opt()])
                return b_out

            # ================= self-attention =================
            with tc.tile_pool(name="ap", bufs=1) as apool:
                b = attention('sa', sa_cls, msk_sa, apool)
                residual_add(b, "sa_res")
                # ============= cross-attention =============
                b = attention('ca', ca_cls, msk_ca, apool)
                residual_add(b, "ca_res")
            # ================= MLP =================
            rmsnorm("mlp_norm")
            mpool_ctx = tc.tile_pool(name="mp", bufs=1)
            mpool = mpool_ctx.__enter__()
            gact = [mpool.tile([128, S], BF, name=f"gact{f}", tag=f"gact{f}",
                               bufs=1) for f in range(FFT)]
            with nc.named_scope("mlp_up"):
                for f in range(FFT):
                    wg = wpool.tile([128, C], BF, tag="wbig")
                    nc.sync.dma_start(wg[:], d_w['wg'].ap()[f])
                    wu = wpool.tile([128, C], BF, tag="wbig")
                    nc.sync.dma_start(wu[:], d_w['wu'].ap()[f])
                    for sc in range(ST):
                        s0 = slice(sc * 512, (sc + 1) * 512)
                        pg = psA.tile([128, 512], F32, tag="acc")
                        pu = psA.tile([128, 512], F32, tag="acc")
                        for cc in range(CT):
                            nc.tensor.matmul(pg[:], wg[:, cc * 128:(cc + 1) * 128],
                                             ht[cc][:, s0],
                                             start=(cc == 0), stop=(cc == CT - 1))
                        for cc in range(CT):
                            nc.tensor.matmul(pu[:], wu[:, cc * 128:(cc + 1) * 128],
                                             ht[cc][:, s0],
                                             start=(cc == 0), stop=(cc == CT - 1))
                        gs = spool.tile([128, 512], BF, tag="lin")
                        nc.scalar.activation(gs[:], pg[:], AF.Silu)
                        nc.vector.tensor_tensor(gact[f][:, s0], gs[:], pu[:],
                                                op=MULT)
            b_in = dpool.tile([C, S], BF, name="bin_mlp", tag="bin_mlp")
            with nc.named_scope("mlp_down"):
                for cc in range(CT):
                    wd = wpool.tile([128, FFC], BF, tag="wbig")
                    nc.sync.dma_start(wd[:], d_w['wd'].ap()[cc])
                    osb = opool.tile([128, S], BF, tag="obuf")
                    for sc in range(ST):
                        s0 = slice(sc * 512, (sc + 1) * 512)
                        ps = psA.tile([128, 512], F32, tag="acc")
                        for f in range(FFT):
                            nc.tensor.matmul(ps[:], wd[:, f * 128:(f + 1) * 128],
                                             gact[f][:, s0],
                                             start=(f == 0), stop=(f == FFT - 1))
                        nc.scalar.activation(osb[:, s0], ps[:], AF.Copy)
                    nc.sync.dma_start(b_in[cc * 128:(cc + 1) * 128, :], osb[:])
            b_out = dpool.tile([C, S], BF, name="bout_mlp", tag="bout_mlp")
            nc.gpsimd.collective_compute(
                "AllReduce", ADD, replica_groups=REPLICA_GROUPS,
                ins=[b_in[:].opt()], outs=[b_out[:].opt()])
            residual_add(b_out, "mlp_res", final=True)
            mpool_ctx.__exit__(None, None, None)

    nc.compile()
    return nc


# ---------------------------------------------------------------- entry
def _mask_sig(cls, pat):
    return (tuple(tuple(row) for row in cls), pat.tobytes())


def kernel(**inputs) -> np.ndarray:
    shared, per_rank, sa_cls, ca_cls = _prep_host(inputs)
    nb_sa, nb_ca = shared['mask_sa'].shape[0], shared['mask_ca'].shape[0]

    key = (_mask_sig(sa_cls, shared['mask_sa']),
           _mask_sig(ca_cls, shared['mask_ca']))
    if key not in _CACHE:
        _CACHE[key] = _build(sa_cls, ca_cls, nb_sa, nb_ca)
    nc = _CACHE[key]

    x = np.asarray(inputs['x'], dtype=np.float32)
    in_maps = []
    for core in range(NCORES):
        g, r = core // TPG, core % TPG
        m = dict(shared)
        m['x'] = np.ascontiguousarray(x[g])
        m.update(per_rank[r])
        in_maps.append(m)

    res = run_bass_kernel_spmd(nc, in_maps, core_ids=list(range(NCORES)))
    out = np.stack([res.results[0]['out'], res.results[TPG]['out']], axis=0)
    return out.astype(np.float32)


# revision 18
# speedup vs baseline: 1.0442x; 1.0442x over previous
"""Self-contained Trainium2 Bass kernel for nn_ANEDecoderLayer (ANE decoder layer).

Shapes (hardcoded): B=2, C=2048, S=1024, H=16, HD=128, FF=8192, fp32 I/O.

Sharding: 8-way tensor-parallel with batch phase-interleaving.
  Every core holds 2 heads and FF/8=1024 ff channels, and processes BOTH
  batches as two phase-shifted streams: while batch-0's AllReduce is in
  flight, the PE computes batch-1's next phase, so all but the final
  collective hide behind compute. AllReduces are bf16 over all 8 cores.

Host-side preprocessing:
  - RMSNorm weights folded into the following matmul weights.
  - Weights pre-transposed, pre-tiled for lhsT layout, cast to bf16.
  - sin_k/cos_k pre-scaled by 1/sqrt(HD) (folds attention scale into K).
  - kv cache scatter (kv_write_idx) folded into a row-permutation of the mask.
  - additive masks converted to multiplicative exp(mask) tile patterns with
    per-tile classification (all-ones -> no op, all-zero -> tile skipped,
    else multiply by a deduplicated pattern tile).
  - x shipped as bf16 (residual stream held in bf16 SBUF).

Compute: matmuls in bf16 (fp32 PSUM accumulation), RMSNorm statistics in
fp32, softmax without max-subtraction (scores are O(5); exp(-1e9)=0 handled
by tile skipping).
"""

import numpy as np
import ml_dtypes

import concourse.mybir as mybir
import concourse.tile as tile
from concourse import bacc
from concourse.bass_utils import run_bass_kernel_spmd

# ---------------------------------------------------------------- constants
B, C, S, H, HD, FF = 2, 2048, 1024, 16, 128, 8192
EPS = 1e-5
SCALE = 1.0 / float(np.sqrt(HD))

NCORES = 8
TP = 8                       # tensor-parallel degree
HPC = H // TP                # heads per core = 2
OCA = HPC * HD               # attention out-channels per core = 256
FFC = FF // TP               # ff channels per core = 1024

CT = C // 128                # 16 c-chunks
ST = S // 512                # 2 s-chunks of 512
KT = S // 128                # 8 k-chunks
FFT = FFC // 128             # 8 ff-chunks per core

F32 = mybir.dt.float32
BF = mybir.dt.bfloat16
AF = mybir.ActivationFunctionType
MULT = mybir.AluOpType.mult
ADD = mybir.AluOpType.add
BF_NP = ml_dtypes.bfloat16

REPLICA_GROUPS = [[0, 1, 2, 3, 4, 5, 6, 7]]

_CACHE: dict = {}


# ---------------------------------------------------------------- host prep
def _pack_lhsT(wT: np.ndarray) -> np.ndarray:
    """wT: (K, M) contraction-major weight. Returns (M//128, 128, K) bf16 where
    pack[m][p, kc*128+f] = wT[kc*128+p, m*128+f]; a DMA of pack[m] gives an
    SBUF tile whose slice [:, kc*128:(kc+1)*128] is the lhsT for contraction
    chunk kc -> output chunk m."""
    K, M = wT.shape
    Kt, Mt = K // 128, M // 128
    t = wT.reshape(Kt, 128, Mt, 128)              # [kc, p, m, f]
    t = t.transpose(2, 1, 0, 3).reshape(Mt, 128, K)
    return np.ascontiguousarray(t.astype(BF_NP))


def _classify_mask(mask_eff: np.ndarray):
    """mask_eff: (S, S) additive mask, (k, q) orientation. Returns
    (cls, patterns): cls[qc][kc] in {'c' (clean), 's' (skip), int idx};
    patterns: (NB, 128, 512) bf16 multiplicative tiles."""
    mm = np.exp(np.minimum(mask_eff.astype(np.float64), 0.0)).astype(np.float32)
    if mask_eff.max() > 0:
        mm = np.exp(mask_eff.astype(np.float64)).astype(np.float32)
    patterns = []
    keys = {}
    cls = [[None] * KT for _ in range(ST)]
    for qc in range(ST):
        for kc in range(KT):
            sub = mm[kc * 128:(kc + 1) * 128, qc * 512:(qc + 1) * 512]
            if np.all(sub == 1.0):
                cls[qc][kc] = 'c'
            elif np.all(sub == 0.0):
                cls[qc][kc] = 's'
            else:
                kb = sub.tobytes()
                if kb not in keys:
                    keys[kb] = len(patterns)
                    patterns.append(sub.astype(BF_NP))
                cls[qc][kc] = keys[kb]
    if patterns:
        pat = np.stack(patterns)
    else:
        pat = np.zeros((1, 128, 512), BF_NP)
    return cls, pat


def _prep_host(inputs):
    """Returns (shared_map, per_rank_maps, sa_cls, ca_cls)."""
    g = lambda k: np.asarray(inputs[k], dtype=np.float32)

    sinq = np.ascontiguousarray(g('sin_q').reshape(HD, S))
    cosq = np.ascontiguousarray(g('cos_q').reshape(HD, S))
    sink = np.ascontiguousarray(g('sin_k').reshape(HD, S) * SCALE)
    cosk = np.ascontiguousarray(g('cos_k').reshape(HD, S) * SCALE)

    idx = np.asarray(inputs['kv_write_idx']).astype(np.int64)
    if not np.array_equal(np.sort(idx), np.arange(S)):
        raise NotImplementedError("kv_write_idx must be a permutation of arange(S)")
    sa_mask = g('self_attn_mask').reshape(S, S)[idx, :]     # effective (k, q) mask
    ca_mask = g('cross_attn_mask').reshape(S, S)
    sa_cls, sa_pat = _classify_mask(sa_mask)
    ca_cls, ca_pat = _classify_mask(ca_mask)

    P_rot = np.zeros((HD, HD), np.float32)
    P_rot[np.arange(64), np.arange(64, 128)] = -1.0
    P_rot[np.arange(64, 128), np.arange(64)] = 1.0

    x = g('x')
    shared = {
        'x0': x[0].astype(BF_NP), 'x1': x[1].astype(BF_NP),
        'sinq': sinq.astype(BF_NP), 'cosq': cosq.astype(BF_NP),
        'sink': sink.astype(BF_NP), 'cosk': cosk.astype(BF_NP),
        'ones_col': np.ones((128, 1), np.float32),
        'ones_col_bf': np.ones((128, 1), BF_NP),
        'ones_row': np.ones((1, 128), np.float32),
        'ident': np.eye(128).astype(BF_NP),
        'protT': np.ascontiguousarray(P_rot.T).astype(BF_NP),
        'mask_sa': sa_pat, 'mask_ca': ca_pat,
    }

    w_sa, w_ca, w_mlp = g('w_sa'), g('w_ca'), g('w_mlp')
    per_rank = []
    for r in range(TP):
        asl = slice(r * OCA, (r + 1) * OCA)
        fsl = slice(r * FFC, (r + 1) * FFC)
        m = {}
        for t in ('sa', 'ca'):
            wnorm = w_sa if t == 'sa' else w_ca
            for p in ('q', 'k', 'v'):
                W = g(f'w{p}_{t}')[asl, :] * wnorm[None, :]
                m[f'w{p}_{t}'] = _pack_lhsT(np.ascontiguousarray(W.T))
            Wo = g(f'wo_{t}')[:, asl]
            m[f'wo_{t}'] = _pack_lhsT(np.ascontiguousarray(Wo.T))
        for p, key in (('g', 'w_gate'), ('u', 'w_up')):
            W = g(key)[fsl, :] * w_mlp[None, :]
            m[f'w{p}'] = _pack_lhsT(np.ascontiguousarray(W.T))
        Wd = g('w_down')[:, fsl]
        m['wd'] = _pack_lhsT(np.ascontiguousarray(Wd.T))
        per_rank.append(m)

    return shared, per_rank, sa_cls, ca_cls


# ---------------------------------------------------------------- builder
def _build(sa_cls, ca_cls, nb_sa, nb_ca):
    nc = bacc.Bacc("TRN2", target_bir_lowering=False, debug=False,
                   num_devices=NCORES)

    d_x = {b: nc.declare_dram_parameter(f"x{b}", [C, S], BF, isOutput=False)
           for b in range(B)}
    d_tab = {k: nc.declare_dram_parameter(k, [HD, S], BF, isOutput=False)
             for k in ('sinq', 'cosq', 'sink', 'cosk')}
    d_oc = nc.declare_dram_parameter("ones_col", [128, 1], F32, isOutput=False)
    d_ocb = nc.declare_dram_parameter("ones_col_bf", [128, 1], BF, isOutput=False)
    d_or = nc.declare_dram_parameter("ones_row", [1, 128], F32, isOutput=False)
    d_id = nc.declare_dram_parameter("ident", [128, 128], BF, isOutput=False)
    d_pr = nc.declare_dram_parameter("protT", [128, 128], BF, isOutput=False)
    d_msa = nc.declare_dram_parameter("mask_sa", [nb_sa, 128, 512], BF, isOutput=False)
    d_mca = nc.declare_dram_parameter("mask_ca", [nb_ca, 128, 512], BF, isOutput=False)
    d_w = {}
    for t in ('sa', 'ca'):
        for p in ('q', 'k', 'v'):
            d_w[f'w{p}_{t}'] = nc.declare_dram_parameter(
                f'w{p}_{t}', [OCA // 128, 128, C], BF, isOutput=False)
        d_w[f'wo_{t}'] = nc.declare_dram_parameter(
            f'wo_{t}', [CT, 128, OCA], BF, isOutput=False)
    for k in ('wg', 'wu'):
        d_w[k] = nc.declare_dram_parameter(k, [FFT, 128, C], BF, isOutput=False)
    d_w['wd'] = nc.declare_dram_parameter('wd', [CT, 128, FFC], BF, isOutput=False)
    d_out = {b: nc.declare_dram_parameter(f"out{b}", [C, S], F32, isOutput=True)
             for b in range(B)}

    with tile.TileContext(nc) as tc:
        with (
            tc.tile_pool(name="const", bufs=1) as cpool,
            tc.tile_pool(name="xp", bufs=1) as xpool,
            tc.tile_pool(name="hp", bufs=1) as hpool,
            tc.tile_pool(name="ap", bufs=1) as apool,
            tc.tile_pool(name="mp", bufs=1) as mpool,
            tc.tile_pool(name="wb", bufs=3) as wpool,
            tc.tile_pool(name="oo", bufs=2) as opool,
            tc.tile_pool(name="sm", bufs=2) as spool,
            tc.tile_pool(name="dram", bufs=1, space="DRAM") as dpool,
            tc.tile_pool(name="psA", bufs=6, space="PSUM") as psA,
            tc.tile_pool(name="psS", bufs=1, space="PSUM") as psS,
            tc.tile_pool(name="psT", bufs=1, space="PSUM") as psT,
        ):
            # ---------------- constants / tables ----------------
            def ptile(pool, shape, dt, name):
                return pool.tile(shape, dt, name=name, tag=name)

            ones_col = ptile(cpool, [128, 1], F32, "ones_col")
            ones_col_bf = ptile(cpool, [128, 1], BF, "ones_col_bf")
            ones_row = ptile(cpool, [1, 128], F32, "ones_row")
            ident = ptile(cpool, [128, 128], BF, "ident")
            protT = ptile(cpool, [128, 128], BF, "protT")
            eps_t = ptile(cpool, [128, 1], F32, "eps_t")
            nc.sync.dma_start(ones_col[:], d_oc.ap())
            nc.sync.dma_start(ones_col_bf[:], d_ocb.ap())
            nc.sync.dma_start(ones_row[:], d_or.ap())
            nc.sync.dma_start(ident[:], d_id.ap())
            nc.sync.dma_start(protT[:], d_pr.ap())
            nc.vector.memset(eps_t[:], EPS)
            tabs = {}
            for k in d_tab:
                tabs[k] = ptile(cpool, [HD, S], BF, f"tab_{k}")
                nc.sync.dma_start(tabs[k][:], d_tab[k].ap())
            used_sa = {c for row in sa_cls for c in row if isinstance(c, int)}
            used_ca = {c for row in ca_cls for c in row if isinstance(c, int)}
            msk_sa, msk_ca = {}, {}
            for i in sorted(used_sa):
                msk_sa[i] = ptile(cpool, [128, 512], BF, f"msa{i}")
                nc.sync.dma_start(msk_sa[i][:], d_msa.ap()[i])
            for i in sorted(used_ca):
                msk_ca[i] = ptile(cpool, [128, 512], BF, f"mca{i}")
                nc.sync.dma_start(msk_ca[i][:], d_mca.ap()[i])

            # ---------------- residual streams (bf16) ----------------
            xt = {}
            for b in range(B):
                xt[b] = [ptile(xpool, [128, S], BF, f"x{b}_{cc}")
                         for cc in range(CT)]
                for cc in range(CT):
                    nc.sync.dma_start(xt[b][cc][:],
                                      d_x[b].ap()[cc * 128:(cc + 1) * 128, :])
            ht = [ptile(hpool, [128, S], BF, f"h{cc}") for cc in range(CT)]

            # ---------------- helpers ----------------
            def rmsnorm(b, scope):
                """ht = xt[b] * rsqrt(mean_c(xt^2) + eps)."""
                with nc.named_scope(scope):
                    for sc in range(ST):
                        s0 = slice(sc * 512, (sc + 1) * 512)
                        ss = psS.tile([1, 512], F32, tag="sum")
                        for cc in range(CT):
                            sq = spool.tile([128, 512], F32, tag="sq")
                            nc.scalar.activation(sq[:], xt[b][cc][:, s0],
                                                 AF.Square)
                            nc.tensor.matmul(ss[:], ones_col[:], sq[:],
                                             start=(cc == 0), stop=(cc == CT - 1))
                        rs = spool.tile([1, 512], F32, tag="rs")
                        nc.scalar.activation(rs[:], ss[:], AF.Sqrt,
                                             bias=eps_t[:1, :], scale=1.0 / C)
                        rr = spool.tile([1, 512], F32, tag="rr")
                        nc.vector.reciprocal_approx_fast(rr[:], rs[:])
                        bc = psA.tile([128, 512], F32, tag="acc")
                        nc.tensor.matmul(bc[:], ones_row[:], rr[:],
                                         start=True, stop=True)
                        for cc in range(CT):
                            nc.vector.tensor_tensor(ht[cc][:, s0],
                                                    xt[b][cc][:, s0],
                                                    bc[:], op=MULT)

            def residual(b, b_prev, scope):
                """xt[b] += AR result (bf16 dram (C,S))."""
                with nc.named_scope(scope):
                    for cc in range(CT):
                        ar = opool.tile([128, S], BF, tag="ar")
                        nc.sync.dma_start(ar[:],
                                          b_prev[cc * 128:(cc + 1) * 128, :])
                        nc.vector.tensor_tensor(xt[b][cc][:], xt[b][cc][:],
                                                ar[:], op=ADD)

            def final_res(b, halves, scope):
                """out[b] = xt[b] + AR halves (f32)."""
                with nc.named_scope(scope):
                    for sc in range(ST):
                        s0 = slice(sc * 512, (sc + 1) * 512)
                        for cc in range(CT):
                            ar = opool.tile([128, 512], BF, tag="arf")
                            nc.sync.dma_start(
                                ar[:], halves[sc][cc * 128:(cc + 1) * 128, :])
                            ot = opool.tile([128, 512], F32, tag="obuf")
                            nc.vector.tensor_tensor(ot[:], xt[b][cc][:, s0],
                                                    ar[:], op=ADD)
                            nc.sync.dma_start(
                                d_out[b].ap()[cc * 128:(cc + 1) * 128, s0],
                                ot[:])

            def attention(t, b, cls, msk, b_prev):
                """One attention block for batch b. Applies the previous
                block's residual lazily (so its AR overlapped the other
                batch's compute). Returns this block's AR output."""
                if b_prev is not None:
                    residual(b, b_prev, f"{t}{b}_res")
                rmsnorm(b, f"{t}{b}_norm")
                qk_rope, vT = {}, {}
                with nc.named_scope(f"{t}{b}_qkv"):
                    for p in ('k', 'v', 'q'):
                        for oc in range(HPC):
                            if p == 'v':
                                vT[oc] = apool.tile([128, S], BF,
                                                    name=f"vT{t}{b}{oc}",
                                                    tag=f"vT{oc}", bufs=1)
                            else:
                                qk_rope[(p, oc)] = apool.tile(
                                    [128, S], BF, name=f"{p}r{t}{b}{oc}",
                                    tag=f"{p}r{oc}", bufs=1)
                            wsb = wpool.tile([128, C], BF, tag="wbig")
                            nc.sync.dma_start(wsb[:], d_w[f'w{p}_{t}'].ap()[oc])
                            for sc in range(ST):
                                s0 = slice(sc * 512, (sc + 1) * 512)
                                ps = psA.tile([128, 512], F32, tag="acc")
                                for cc in range(CT):
                                    nc.tensor.matmul(
                                        ps[:], wsb[:, cc * 128:(cc + 1) * 128],
                                        ht[cc][:, s0],
                                        start=(cc == 0), stop=(cc == CT - 1))
                                if p == 'v':
                                    vsb = spool.tile([128, 512], BF, tag="lin")
                                    nc.scalar.activation(vsb[:], ps[:], AF.Copy)
                                    for j in range(4):
                                        kc = sc * 4 + j
                                        tp = psT.tile([128, 128], BF, tag="tr")
                                        nc.tensor.transpose(
                                            tp[:], vsb[:, j * 128:(j + 1) * 128],
                                            ident[:])
                                        nc.scalar.activation(
                                            vT[oc][:, kc * 128:(kc + 1) * 128],
                                            tp[:], AF.Copy)
                                else:
                                    lin = spool.tile([128, 512], BF, tag="lin")
                                    nc.scalar.activation(lin[:], ps[:], AF.Copy)
                                    rot = psA.tile([128, 512], F32, tag="acc")
                                    nc.tensor.matmul(rot[:], protT[:], lin[:],
                                                     start=True, stop=True)
                                    sin = tabs['sinq' if p == 'q' else 'sink']
                                    cos = tabs['cosq' if p == 'q' else 'cosk']
                                    dst = qk_rope[(p, oc)]
                                    nc.vector.tensor_tensor(
                                        dst[:, s0], lin[:], cos[:, s0], op=MULT)
                                    s2 = spool.tile([128, 512], BF, tag="rsc")
                                    nc.vector.tensor_tensor(
                                        s2[:], rot[:], sin[:, s0], op=MULT)
                                    nc.vector.tensor_tensor(
                                        dst[:, s0], dst[:, s0], s2[:], op=ADD)
                att = [apool.tile([128, S], BF, name=f"att{t}{b}{oc}",
                                  tag=f"att{oc}", bufs=1) for oc in range(HPC)]
                with nc.named_scope(f"{t}{b}_attn"):
                    for oc in range(HPC):
                        qr, kr = qk_rope[('q', oc)], qk_rope[('k', oc)]
                        for qc in range(ST):
                            s0 = slice(qc * 512, (qc + 1) * 512)
                            valid = [kc for kc in range(KT) if cls[qc][kc] != 's']
                            probs = {}
                            for kc in valid:
                                sp = psA.tile([128, 512], F32, tag="acc")
                                nc.tensor.matmul(
                                    sp[:], kr[:, kc * 128:(kc + 1) * 128],
                                    qr[:, s0], start=True, stop=True)
                                pt = apool.tile([128, 512], BF, tag="probs",
                                                bufs=8)
                                nc.scalar.activation(pt[:], sp[:], AF.Exp)
                                if cls[qc][kc] != 'c':
                                    nc.vector.tensor_tensor(
                                        pt[:], pt[:], msk[cls[qc][kc]][:],
                                        op=MULT)
                                probs[kc] = pt
                            dn = psS.tile([1, 512], F32, tag="sum")
                            for i, kc in enumerate(valid):
                                nc.tensor.matmul(dn[:], ones_col_bf[:],
                                                 probs[kc][:],
                                                 start=(i == 0),
                                                 stop=(i == len(valid) - 1))
                            ra = spool.tile([1, 512], F32, tag="ra")
                            nc.vector.reciprocal_approx_fast(ra[:], dn[:])
                            rb = psA.tile([128, 512], F32, tag="acc")
                            nc.tensor.matmul(rb[:], ones_row[:], ra[:],
                                             start=True, stop=True)
                            # DVE reads only one PSUM input; stage bcast in SBUF
                            rbs = spool.tile([128, 512], BF, tag="rbs")
                            nc.scalar.activation(rbs[:], rb[:], AF.Copy)
                            pa = psA.tile([128, 512], F32, tag="acc")
                            for i, kc in enumerate(valid):
                                nc.tensor.matmul(
                                    pa[:], vT[oc][:, kc * 128:(kc + 1) * 128],
                                    probs[kc][:],
                                    start=(i == 0), stop=(i == len(valid) - 1))
                            nc.vector.tensor_tensor(att[oc][:, s0], pa[:],
                                                    rbs[:], op=MULT)
                b_in = dpool.tile([C, S], BF, name=f"bin_{t}{b}",
                                  tag=f"bin_{t}{b}")
                b_out = dpool.tile([C, S], BF, name=f"bout_{t}{b}",
                                   tag=f"bout_{t}{b}")
                with nc.named_scope(f"{t}{b}_wo"):
                    for cc in range(CT):
                        wsb = wpool.tile([128, OCA], BF, tag="wsm")
                        nc.sync.dma_start(wsb[:], d_w[f'wo_{t}'].ap()[cc])
                        osb = opool.tile([128, S], BF, tag="osb")
                        for sc in range(ST):
                            s0 = slice(sc * 512, (sc + 1) * 512)
                            ps = psA.tile([128, 512], F32, tag="acc")
                            for ac in range(HPC):
                                nc.tensor.matmul(
                                    ps[:], wsb[:, ac * 128:(ac + 1) * 128],
                                    att[ac][:, s0],
                                    start=(ac == 0), stop=(ac == HPC - 1))
                            nc.scalar.activation(osb[:, s0], ps[:], AF.Copy)
                        nc.sync.dma_start(b_in[cc * 128:(cc + 1) * 128, :],
                                          osb[:])
                nc.gpsimd.collective_compute(
                    "AllReduce", ADD, replica_groups=REPLICA_GROUPS,
                    ins=[b_in[:].opt()], outs=[b_out[:].opt()])
                return b_out

            gact = [ptile(mpool, [128, S], BF, f"gact{f}") for f in range(FFT)]

            def mlp(b, b_prev):
                """MLP block for batch b; returns AR output halves."""
                residual(b, b_prev, f"mlp{b}_res")
                rmsnorm(b, f"mlp{b}_norm")
                b_in = [dpool.tile([C, 512], BF, name=f"bin_mlp{b}{h}",
                                   tag=f"bin_mlp{b}{h}") for h in range(ST)]
                b_out = [dpool.tile([C, 512], BF, name=f"bout_mlp{b}{h}",
                                    tag=f"bout_mlp{b}{h}") for h in range(ST)]
                with nc.named_scope(f"mlp{b}_up"):
                    for f in range(FFT):
                        wg = wpool.tile([128, C], BF, tag="wbig")
                        nc.sync.dma_start(wg[:], d_w['wg'].ap()[f])
                        wu = wpool.tile([128, C], BF, tag="wbig")
                        nc.sync.dma_start(wu[:], d_w['wu'].ap()[f])
                        for sc in range(ST):
                            s0 = slice(sc * 512, (sc + 1) * 512)
                            pg = psA.tile([128, 512], F32, tag="acc")
                            for cc in range(CT):
                                nc.tensor.matmul(
                                    pg[:], wg[:, cc * 128:(cc + 1) * 128],
                                    ht[cc][:, s0],
                                    start=(cc == 0), stop=(cc == CT - 1))
                            pu = psA.tile([128, 512], F32, tag="acc")
                            for cc in range(CT):
                                nc.tensor.matmul(
                                    pu[:], wu[:, cc * 128:(cc + 1) * 128],
                                    ht[cc][:, s0],
                                    start=(cc == 0), stop=(cc == CT - 1))
                            gs = spool.tile([128, 512], BF, tag="lin")
                            nc.scalar.activation(gs[:], pg[:], AF.Silu)
                            nc.vector.tensor_tensor(gact[f][:, s0], gs[:],
                                                    pu[:], op=MULT)
                with nc.named_scope(f"mlp{b}_down"):
                    for sc in range(ST):
                        s0 = slice(sc * 512, (sc + 1) * 512)
                        for cc in range(CT):
                            wd = wpool.tile([128, FFC], BF, tag="wsm")
                            nc.sync.dma_start(wd[:], d_w['wd'].ap()[cc])
                            ps = psA.tile([128, 512], F32, tag="acc")
                            for f in range(FFT):
                                nc.tensor.matmul(
                                    ps[:], wd[:, f * 128:(f + 1) * 128],
                                    gact[f][:, s0],
                                    start=(f == 0), stop=(f == FFT - 1))
                            osb = opool.tile([128, 512], BF, tag="osbh")
                            nc.scalar.activation(osb[:], ps[:], AF.Copy)
                            nc.sync.dma_start(
                                b_in[sc][cc * 128:(cc + 1) * 128, :], osb[:])
                        nc.gpsimd.collective_compute(
                            "AllReduce", ADD, replica_groups=REPLICA_GROUPS,
                            ins=[b_in[sc][:].opt()], outs=[b_out[sc][:].opt()])
                return b_out

            # ============ phase-interleaved batch schedule ============
            b_sa = {}
            for b in range(B):
                b_sa[b] = attention('sa', b, sa_cls, msk_sa, None)
            b_ca = {}
            for b in range(B):
                b_ca[b] = attention('ca', b, ca_cls, msk_ca, b_sa[b])
            b_mlp = {}
            for b in range(B):
                b_mlp[b] = mlp(b, b_ca[b])
            for b in range(B):
                final_res(b, b_mlp[b], f"fin{b}")

    nc.compile()
    return nc


# ---------------------------------------------------------------- entry
def _mask_sig(cls, pat):
    return (tuple(tuple(row) for row in cls), pat.tobytes())


def kernel(**inputs) -> np.ndarray:
    shared, per_rank, sa_cls, ca_cls = _prep_host(inputs)
    nb_sa, nb_ca = shared['mask_sa'].shape[0], shared['mask_ca'].shape[0]

    key = (_mask_sig(sa_cls, shared['mask_sa']),
           _mask_sig(ca_cls, shared['mask_ca']))
    if key not in _CACHE:
        _CACHE[key] = _build(sa_cls, ca_cls, nb_sa, nb_ca)
    nc = _CACHE[key]

    in_maps = []
    for core in range(NCORES):
        m = dict(shared)
        m.update(per_rank[core])
        in_maps.append(m)

    res = run_bass_kernel_spmd(nc, in_maps, core_ids=list(range(NCORES)))
    out = np.stack([res.results[0]['out0'], res.results[0]['out1']], axis=0)
    return out.astype(np.float32)


# revision 19
# speedup vs baseline: 1.1162x; 1.0690x over previous
"""Self-contained Trainium2 Bass kernel for nn_ANEDecoderLayer (ANE decoder layer).

Shapes (hardcoded): B=2, C=2048, S=1024, H=16, HD=128, FF=8192, fp32 I/O.

Sharding: hybrid batch(2) x tensor-parallel(4) over 8 NeuronCores.
  core = g*4 + r:  g = batch index, r = TP rank.
  Within each group of 4 cores: heads sharded 4/core, d_ff sharded 2048/core.
  Block outputs (row-split Wo / W_down partial sums) are AllReduced in bf16
  within each 4-core group: replica_groups [[0,1,2,3],[4,5,6,7]].

Host-side preprocessing:
  - RMSNorm weights folded into the following matmul weights.
  - Weights pre-transposed, pre-tiled for lhsT layout, cast to bf16.
  - sin_k/cos_k pre-scaled by 1/sqrt(HD) (folds attention scale into K).
  - kv cache scatter (kv_write_idx) folded into a row-permutation of the mask.
  - additive masks converted to multiplicative exp(mask) tile patterns with
    per-tile classification (all-ones -> no op, all-zero -> tile skipped,
    else multiply by a deduplicated pattern tile).

Compute: matmuls in bf16 (fp32 PSUM accumulation), RMSNorm statistics in
fp32, softmax without max-subtraction (scores are O(5); exp(-1e9)=0 handled
by tile skipping), residual stream held in fp32 in SBUF.
"""

import numpy as np
import ml_dtypes

import concourse.mybir as mybir
import concourse.tile as tile
from concourse import bacc
from concourse.bass_utils import run_bass_kernel_spmd

# ---------------------------------------------------------------- constants
B, C, S, H, HD, FF = 2, 2048, 1024, 16, 128, 8192
EPS = 1e-5
SCALE = 1.0 / float(np.sqrt(HD))

NCORES = 8
TPG = 4                      # tensor-parallel group size
HPC = H // TPG               # heads per core = 4
OCA = HPC * HD               # attention out-channels per core = 512
FFC = FF // TPG              # ff channels per core = 2048

CT = C // 128                # 16 c-chunks
ST = S // 512                # 2 s-chunks of 512
KT = S // 128                # 8 k-chunks
FFT = FFC // 128             # 16 ff-chunks per core

F32 = mybir.dt.float32
BF = mybir.dt.bfloat16
AF = mybir.ActivationFunctionType
MULT = mybir.AluOpType.mult
ADD = mybir.AluOpType.add
BF_NP = ml_dtypes.bfloat16

REPLICA_GROUPS = [[0, 1, 2, 3], [4, 5, 6, 7]]

_CACHE: dict = {}


# ---------------------------------------------------------------- host prep
def _pack_lhsT(wT: np.ndarray) -> np.ndarray:
    """wT: (K, M) contraction-major weight. Returns (M//128, 128, K) bf16 where
    pack[m][p, kc*128+f] = wT[kc*128+p, m*128+f]; a DMA of pack[m] gives an
    SBUF tile whose slice [:, kc*128:(kc+1)*128] is the lhsT for contraction
    chunk kc -> output chunk m."""
    K, M = wT.shape
    Kt, Mt = K // 128, M // 128
    t = wT.reshape(Kt, 128, Mt, 128)              # [kc, p, m, f]
    t = t.transpose(2, 1, 0, 3).reshape(Mt, 128, K)
    return np.ascontiguousarray(t.astype(BF_NP))


def _classify_mask(mask_eff: np.ndarray):
    """mask_eff: (S, S) additive mask, (k, q) orientation. Returns
    (cls, patterns): cls[qc][kc] in {'c' (clean), 's' (skip), int idx};
    patterns: (NB, 128, 512) bf16 multiplicative tiles."""
    mm = np.exp(np.minimum(mask_eff.astype(np.float64), 0.0)).astype(np.float32)
    # positive masks would overflow exp; reference masks are <= 0
    if mask_eff.max() > 0:
        mm = np.exp(mask_eff.astype(np.float64)).astype(np.float32)
    patterns = []
    keys = {}
    cls = [[None] * KT for _ in range(ST)]
    for qc in range(ST):
        for kc in range(KT):
            sub = mm[kc * 128:(kc + 1) * 128, qc * 512:(qc + 1) * 512]
            if np.all(sub == 1.0):
                cls[qc][kc] = 'c'
            elif np.all(sub == 0.0):
                cls[qc][kc] = 's'
            else:
                kb = sub.tobytes()
                if kb not in keys:
                    keys[kb] = len(patterns)
                    patterns.append(sub.astype(BF_NP))
                cls[qc][kc] = keys[kb]
    if patterns:
        pat = np.stack(patterns)
    else:
        pat = np.zeros((1, 128, 512), BF_NP)
    return cls, pat


def _prep_host(inputs):
    """Returns (shared_map, per_rank_maps, sa_cls, ca_cls)."""
    g = lambda k: np.asarray(inputs[k], dtype=np.float32)

    sinq = np.ascontiguousarray(g('sin_q').reshape(HD, S))
    cosq = np.ascontiguousarray(g('cos_q').reshape(HD, S))
    sink = np.ascontiguousarray(g('sin_k').reshape(HD, S) * SCALE)
    cosk = np.ascontiguousarray(g('cos_k').reshape(HD, S) * SCALE)

    idx = np.asarray(inputs['kv_write_idx']).astype(np.int64)
    if not np.array_equal(np.sort(idx), np.arange(S)):
        raise NotImplementedError("kv_write_idx must be a permutation of arange(S)")
    sa_mask = g('self_attn_mask').reshape(S, S)[idx, :]     # effective (k, q) mask
    ca_mask = g('cross_attn_mask').reshape(S, S)
    sa_cls, sa_pat = _classify_mask(sa_mask)
    ca_cls, ca_pat = _classify_mask(ca_mask)

    P_rot = np.zeros((HD, HD), np.float32)
    P_rot[np.arange(64), np.arange(64, 128)] = -1.0
    P_rot[np.arange(64, 128), np.arange(64)] = 1.0

    shared = {
        'sinq': sinq.astype(BF_NP), 'cosq': cosq.astype(BF_NP),
        'sink': sink.astype(BF_NP), 'cosk': cosk.astype(BF_NP),
        'ones_col': np.ones((128, 1), np.float32),
        'ones_col_bf': np.ones((128, 1), BF_NP),
        'ones_row': np.ones((1, 128), np.float32),
        'ident': np.eye(128).astype(BF_NP),
        'protT': np.ascontiguousarray(P_rot.T).astype(BF_NP),
        'mask_sa': sa_pat, 'mask_ca': ca_pat,
    }

    w_sa, w_ca, w_mlp = g('w_sa'), g('w_ca'), g('w_mlp')
    per_rank = []
    for r in range(TPG):
        asl = slice(r * OCA, (r + 1) * OCA)
        fsl = slice(r * FFC, (r + 1) * FFC)
        m = {}
        for tag, wn, on in (('sa', 'w_sa', None), ('ca', 'w_ca', None)):
            wnorm = w_sa if tag == 'sa' else w_ca
            for p in ('q', 'k', 'v'):
                W = g(f'w{p}_{tag}')[asl, :] * wnorm[None, :]
                m[f'w{p}_{tag}'] = _pack_lhsT(np.ascontiguousarray(W.T))
            Wo = g(f'wo_{tag}')[:, asl]
            m[f'wo_{tag}'] = _pack_lhsT(np.ascontiguousarray(Wo.T))
        for p, key in (('g', 'w_gate'), ('u', 'w_up')):
            W = g(key)[fsl, :] * w_mlp[None, :]
            m[f'w{p}'] = _pack_lhsT(np.ascontiguousarray(W.T))
        Wd = g('w_down')[:, fsl]
        m['wd'] = _pack_lhsT(np.ascontiguousarray(Wd.T))
        per_rank.append(m)

    return shared, per_rank, sa_cls, ca_cls


# ---------------------------------------------------------------- builder
def _build(sa_cls, ca_cls, nb_sa, nb_ca):
    nc = bacc.Bacc("TRN2", target_bir_lowering=False, debug=False,
                   num_devices=NCORES)

    d_x = nc.declare_dram_parameter("x", [C, S], BF, isOutput=False)
    d_tab = {k: nc.declare_dram_parameter(k, [HD, S], BF, isOutput=False)
             for k in ('sinq', 'cosq', 'sink', 'cosk')}
    d_oc = nc.declare_dram_parameter("ones_col", [128, 1], F32, isOutput=False)
    d_ocb = nc.declare_dram_parameter("ones_col_bf", [128, 1], BF, isOutput=False)
    d_or = nc.declare_dram_parameter("ones_row", [1, 128], F32, isOutput=False)
    d_id = nc.declare_dram_parameter("ident", [128, 128], BF, isOutput=False)
    d_pr = nc.declare_dram_parameter("protT", [128, 128], BF, isOutput=False)
    d_msa = nc.declare_dram_parameter("mask_sa", [nb_sa, 128, 512], BF, isOutput=False)
    d_mca = nc.declare_dram_parameter("mask_ca", [nb_ca, 128, 512], BF, isOutput=False)
    d_w = {}
    for t in ('sa', 'ca'):
        for p in ('q', 'k', 'v'):
            d_w[f'w{p}_{t}'] = nc.declare_dram_parameter(
                f'w{p}_{t}', [OCA // 128, 128, C], BF, isOutput=False)
        d_w[f'wo_{t}'] = nc.declare_dram_parameter(
            f'wo_{t}', [CT, 128, OCA], BF, isOutput=False)
    for k in ('wg', 'wu', 'wd'):
        kdim = C if k != 'wd' else FFC
        d_w[k] = nc.declare_dram_parameter(k, [FFT, 128, kdim], BF, isOutput=False)
    d_out = nc.declare_dram_parameter("out", [C, S], F32, isOutput=True)

    with tile.TileContext(nc) as tc:
        with (
            tc.tile_pool(name="const", bufs=1) as cpool,
            tc.tile_pool(name="xp", bufs=1) as xpool,
            tc.tile_pool(name="hp", bufs=1) as hpool,
            tc.tile_pool(name="wb", bufs=3) as wpool,
            tc.tile_pool(name="oo", bufs=2) as opool,
            tc.tile_pool(name="sm", bufs=2) as spool,
            tc.tile_pool(name="dram", bufs=1, space="DRAM") as dpool,
            tc.tile_pool(name="psA", bufs=5, space="PSUM") as psA,
            tc.tile_pool(name="psS", bufs=1, space="PSUM") as psS,
            tc.tile_pool(name="psT", bufs=1, space="PSUM") as psT,
        ):
            # ---------------- constants / tables ----------------
            def ptile(pool, shape, dt, name):
                return pool.tile(shape, dt, name=name, tag=name)

            ones_col = ptile(cpool, [128, 1], F32, "ones_col")
            ones_col_bf = ptile(cpool, [128, 1], BF, "ones_col_bf")
            ones_row = ptile(cpool, [1, 128], F32, "ones_row")
            ident = ptile(cpool, [128, 128], BF, "ident")
            protT = ptile(cpool, [128, 128], BF, "protT")
            eps_t = ptile(cpool, [128, 1], F32, "eps_t")
            nc.sync.dma_start(ones_col[:], d_oc.ap())
            nc.sync.dma_start(ones_col_bf[:], d_ocb.ap())
            nc.sync.dma_start(ones_row[:], d_or.ap())
            nc.sync.dma_start(ident[:], d_id.ap())
            nc.sync.dma_start(protT[:], d_pr.ap())
            nc.vector.memset(eps_t[:], EPS)
            tabs = {}
            for k in d_tab:
                tabs[k] = ptile(cpool, [HD, S], BF, f"tab_{k}")
                nc.sync.dma_start(tabs[k][:], d_tab[k].ap())
            used_sa = {c for row in sa_cls for c in row if isinstance(c, int)}
            used_ca = {c for row in ca_cls for c in row if isinstance(c, int)}
            msk_sa, msk_ca = {}, {}
            for i in sorted(used_sa):
                msk_sa[i] = ptile(cpool, [128, 512], BF, f"msa{i}")
                nc.sync.dma_start(msk_sa[i][:], d_msa.ap()[i])
            for i in sorted(used_ca):
                msk_ca[i] = ptile(cpool, [128, 512], BF, f"mca{i}")
                nc.sync.dma_start(msk_ca[i][:], d_mca.ap()[i])

            # ---------------- residual stream x ----------------
            xt = [ptile(xpool, [128, S], BF, f"x{cc}") for cc in range(CT)]
            for cc in range(CT):
                nc.sync.dma_start(xt[cc][:], d_x.ap()[cc * 128:(cc + 1) * 128, :])
            ht = [ptile(hpool, [128, S], BF, f"h{cc}") for cc in range(CT)]

            # ---------------- helpers ----------------
            def rmsnorm(scope):
                """ht[:] = xt * rsqrt(mean_c(xt^2) + eps), bf16."""
                with nc.named_scope(scope):
                    for sc in range(ST):
                        ss = psS.tile([1, 512], F32, tag="nsum")
                        for cc in range(CT):
                            sq = spool.tile([128, 512], F32, tag="sq")
                            nc.scalar.activation(
                                sq[:], xt[cc][:, sc * 512:(sc + 1) * 512], AF.Square)
                            nc.tensor.matmul(ss[:], ones_col[:], sq[:],
                                             start=(cc == 0), stop=(cc == CT - 1))
                        rs = spool.tile([1, 512], F32, tag="rs")
                        nc.scalar.activation(rs[:], ss[:], AF.Sqrt,
                                             bias=eps_t[:1, :], scale=1.0 / C)
                        rr = spool.tile([1, 512], F32, tag="rr")
                        nc.vector.reciprocal_approx_fast(rr[:], rs[:])
                        bc = psA.tile([128, 512], F32, tag="acc")
                        nc.tensor.matmul(bc[:], ones_row[:], rr[:],
                                         start=True, stop=True)
                        for cc in range(CT):
                            nc.vector.tensor_tensor(
                                ht[cc][:, sc * 512:(sc + 1) * 512],
                                xt[cc][:, sc * 512:(sc + 1) * 512],
                                bc[:], op=MULT)

            def residual_add(b_out, scope, final=False):
                """xt += AR result (bf16 in dram halves); if final, write out."""
                with nc.named_scope(scope):
                    for cc in range(CT):
                        h, row = cc // (CT // 2), (cc % (CT // 2)) * 128
                        ar = opool.tile([128, S], BF, tag="ar")
                        nc.sync.dma_start(ar[:], b_out[h][row:row + 128, :])
                        if final:
                            ot = opool.tile([128, S], F32, tag="obuf")
                            nc.vector.tensor_tensor(ot[:], xt[cc][:], ar[:], op=ADD)
                            nc.sync.dma_start(
                                d_out.ap()[cc * 128:(cc + 1) * 128, :], ot[:])
                        else:
                            nc.vector.tensor_tensor(xt[cc][:], xt[cc][:], ar[:], op=ADD)

            def attention(t, cls, msk, apool):
                """One attention block (t='sa'|'ca'). Returns bounce-out dram tile."""
                rmsnorm(f"{t}_norm")
                qk_rope = {}
                vT = [apool.tile([128, S], BF, name=f"vT{t}{oc}", tag=f"vT{oc}",
                                 bufs=1) for oc in range(HPC)]
                with nc.named_scope(f"{t}_qkv"):
                    for p in ('q', 'k', 'v'):
                        for oc in range(HPC):
                            if p != 'v':
                                dst = apool.tile([128, S], BF,
                                                 name=f"{p}r{t}{oc}",
                                                 tag=f"{p}r{oc}", bufs=1)
                                qk_rope[(p, oc)] = dst
                            wsb = wpool.tile([128, C], BF, tag="wbig")
                            nc.sync.dma_start(wsb[:], d_w[f'w{p}_{t}'].ap()[oc])
                            pss = [psA.tile([128, 512], F32, tag="acc",
                                            name=f"ps{p}{oc}{j}") for j in range(ST)]
                            for cc in range(CT):
                                wsl = wsb[:, cc * 128:(cc + 1) * 128]
                                for sc in range(ST):
                                    nc.tensor.matmul(
                                        pss[sc][:],
                                        wsl, ht[cc][:, sc * 512:(sc + 1) * 512],
                                        start=(cc == 0), stop=(cc == CT - 1))
                            for sc in range(ST):
                                s0 = slice(sc * 512, (sc + 1) * 512)
                                ps = pss[sc]
                                if p == 'v':
                                    vsb = spool.tile([128, 512], BF, tag="lin")
                                    nc.scalar.activation(vsb[:], ps[:], AF.Copy)
                                    for j in range(4):
                                        kc = sc * 4 + j
                                        tp = psT.tile([128, 128], BF, tag="tr")
                                        nc.tensor.transpose(
                                            tp[:], vsb[:, j * 128:(j + 1) * 128],
                                            ident[:])
                                        nc.scalar.activation(
                                            vT[oc][:, kc * 128:(kc + 1) * 128],
                                            tp[:], AF.Copy)
                                else:
                                    lin = spool.tile([128, 512], BF, tag="lin")
                                    nc.scalar.activation(lin[:], ps[:], AF.Copy)
                                    rot = psA.tile([128, 512], F32, tag="acc")
                                    nc.tensor.matmul(rot[:], protT[:], lin[:],
                                                     start=True, stop=True)
                                    sin = tabs['sinq' if p == 'q' else 'sink']
                                    cos = tabs['cosq' if p == 'q' else 'cosk']
                                    dst = qk_rope[(p, oc)]
                                    nc.vector.tensor_tensor(
                                        dst[:, s0], lin[:], cos[:, s0], op=MULT)
                                    s2 = spool.tile([128, 512], BF, tag="rsc")
                                    nc.vector.tensor_tensor(
                                        s2[:], rot[:], sin[:, s0], op=MULT)
                                    nc.vector.tensor_tensor(
                                        dst[:, s0], dst[:, s0], s2[:], op=ADD)
                att = [apool.tile([128, S], BF, name=f"att{t}{oc}",
                                  tag=f"att{oc}", bufs=1) for oc in range(HPC)]
                with nc.named_scope(f"{t}_attn"):
                    for oc in range(HPC):
                        qr, kr = qk_rope[('q', oc)], qk_rope[('k', oc)]
                        for qc in range(ST):
                            s0 = slice(qc * 512, (qc + 1) * 512)
                            valid = [kc for kc in range(KT) if cls[qc][kc] != 's']
                            probs = {}
                            for kc in valid:
                                sp = psA.tile([128, 512], F32, tag="acc")
                                nc.tensor.matmul(
                                    sp[:], kr[:, kc * 128:(kc + 1) * 128],
                                    qr[:, s0], start=True, stop=True)
                                pt = apool.tile([128, 512], BF, tag="probs",
                                                bufs=12)
                                nc.scalar.activation(pt[:], sp[:], AF.Exp)
                                if cls[qc][kc] != 'c':
                                    nc.vector.tensor_tensor(
                                        pt[:], pt[:], msk[cls[qc][kc]][:], op=MULT)
                                probs[kc] = pt
                            dn = psS.tile([1, 512], F32, tag="dsum")
                            for i, kc in enumerate(valid):
                                nc.tensor.matmul(dn[:], ones_col_bf[:],
                                                 probs[kc][:],
                                                 start=(i == 0),
                                                 stop=(i == len(valid) - 1))
                            ra = spool.tile([1, 512], F32, tag="ra")
                            nc.vector.reciprocal_approx_fast(ra[:], dn[:])
                            rb = psA.tile([128, 512], F32, tag="acc")
                            nc.tensor.matmul(rb[:], ones_row[:], ra[:],
                                             start=True, stop=True)
                            # DVE reads only one PSUM input; stage bcast in SBUF
                            rbs = spool.tile([128, 512], F32, tag="rbs")
                            nc.scalar.activation(rbs[:], rb[:], AF.Copy)
                            pa = psA.tile([128, 512], F32, tag="acc")
                            for i, kc in enumerate(valid):
                                nc.tensor.matmul(
                                    pa[:], vT[oc][:, kc * 128:(kc + 1) * 128],
                                    probs[kc][:],
                                    start=(i == 0), stop=(i == len(valid) - 1))
                            nc.vector.tensor_tensor(att[oc][:, s0], pa[:], rbs[:],
                                                    op=MULT)
                b_in = [dpool.tile([C // 2, S], BF, name=f"bin_{t}{h}",
                                   tag=f"bin_{t}{h}") for h in range(2)]
                b_out = [dpool.tile([C // 2, S], BF, name=f"bout_{t}{h}",
                                    tag=f"bout_{t}{h}") for h in range(2)]
                with nc.named_scope(f"{t}_wo"):
                    for cc in range(CT):
                        wsb = wpool.tile([128, OCA], BF, tag="wsm")
                        nc.sync.dma_start(wsb[:], d_w[f'wo_{t}'].ap()[cc])
                        osb = opool.tile([128, S], BF, tag="obuf")
                        pss = [psA.tile([128, 512], F32, tag="acc",
                                        name=f"pso{cc}{j}") for j in range(ST)]
                        for ac in range(HPC):
                            wsl = wsb[:, ac * 128:(ac + 1) * 128]
                            for sc in range(ST):
                                nc.tensor.matmul(
                                    pss[sc][:], wsl,
                                    att[ac][:, sc * 512:(sc + 1) * 512],
                                    start=(ac == 0), stop=(ac == HPC - 1))
                        for sc in range(ST):
                            s0 = slice(sc * 512, (sc + 1) * 512)
                            nc.scalar.activation(osb[:, s0], pss[sc][:], AF.Copy)
                        h, row = cc // (CT // 2), (cc % (CT // 2)) * 128
                        nc.sync.dma_start(b_in[h][row:row + 128, :], osb[:])
                        if cc == CT // 2 - 1:
                            nc.gpsimd.collective_compute(
                                "AllReduce", ADD, replica_groups=REPLICA_GROUPS,
                                ins=[b_in[0][:].opt()], outs=[b_out[0][:].opt()])
                    nc.gpsimd.collective_compute(
                        "AllReduce", ADD, replica_groups=REPLICA_GROUPS,
                        ins=[b_in[1][:].opt()], outs=[b_out[1][:].opt()])
                return b_out

            # ================= self-attention =================
            with tc.tile_pool(name="ap", bufs=1) as apool:
                b = attention('sa', sa_cls, msk_sa, apool)
                residual_add(b, "sa_res")
                # ============= cross-attention =============
                b = attention('ca', ca_cls, msk_ca, apool)
                residual_add(b, "ca_res")
            # ================= MLP =================
            rmsnorm("mlp_norm")
            mpool_ctx = tc.tile_pool(name="mp", bufs=1)
            mpool = mpool_ctx.__enter__()
            gact = [mpool.tile([128, S], BF, name=f"gact{f}", tag=f"gact{f}",
                               bufs=1) for f in range(FFT)]
            with nc.named_scope("mlp_up"):
                for f in range(FFT):
                    wg = wpool.tile([128, C], BF, tag="wbig")
                    nc.sync.dma_start(wg[:], d_w['wg'].ap()[f])
                    wu = wpool.tile([128, C], BF, tag="wbig")
                    nc.sync.dma_start(wu[:], d_w['wu'].ap()[f])
                    pgs = [psA.tile([128, 512], F32, tag="acc", name=f"pg{f}{j}")
                           for j in range(ST)]
                    pus = [psA.tile([128, 512], F32, tag="acc", name=f"pu{f}{j}")
                           for j in range(ST)]
                    for cc in range(CT):
                        wsl = wg[:, cc * 128:(cc + 1) * 128]
                        for sc in range(ST):
                            nc.tensor.matmul(pgs[sc][:], wsl,
                                             ht[cc][:, sc * 512:(sc + 1) * 512],
                                             start=(cc == 0), stop=(cc == CT - 1))
                    for cc in range(CT):
                        wsl = wu[:, cc * 128:(cc + 1) * 128]
                        for sc in range(ST):
                            nc.tensor.matmul(pus[sc][:], wsl,
                                             ht[cc][:, sc * 512:(sc + 1) * 512],
                                             start=(cc == 0), stop=(cc == CT - 1))
                    for sc in range(ST):
                        s0 = slice(sc * 512, (sc + 1) * 512)
                        gs = spool.tile([128, 512], BF, tag="lin")
                        nc.scalar.activation(gs[:], pgs[sc][:], AF.Silu)
                        nc.vector.tensor_tensor(gact[f][:, s0], gs[:], pus[sc][:],
                                                op=MULT)
            b_in = [dpool.tile([C // 2, S], BF, name=f"bin_mlp{h}",
                               tag=f"bin_mlp{h}") for h in range(2)]
            b_out = [dpool.tile([C // 2, S], BF, name=f"bout_mlp{h}",
                                tag=f"bout_mlp{h}") for h in range(2)]
            with nc.named_scope("mlp_down"):
                for cc in range(CT):
                    wd = wpool.tile([128, FFC], BF, tag="wbig")
                    nc.sync.dma_start(wd[:], d_w['wd'].ap()[cc])
                    osb = opool.tile([128, S], BF, tag="obuf")
                    pss = [psA.tile([128, 512], F32, tag="acc", name=f"pd{cc}{j}")
                           for j in range(ST)]
                    for f in range(FFT):
                        wsl = wd[:, f * 128:(f + 1) * 128]
                        for sc in range(ST):
                            nc.tensor.matmul(pss[sc][:], wsl,
                                             gact[f][:, sc * 512:(sc + 1) * 512],
                                             start=(f == 0), stop=(f == FFT - 1))
                    for sc in range(ST):
                        s0 = slice(sc * 512, (sc + 1) * 512)
                        nc.scalar.activation(osb[:, s0], pss[sc][:], AF.Copy)
                    h, row = cc // (CT // 2), (cc % (CT // 2)) * 128
                    nc.sync.dma_start(b_in[h][row:row + 128, :], osb[:])
                    if cc == CT // 2 - 1:
                        nc.gpsimd.collective_compute(
                            "AllReduce", ADD, replica_groups=REPLICA_GROUPS,
                            ins=[b_in[0][:].opt()], outs=[b_out[0][:].opt()])
                nc.gpsimd.collective_compute(
                    "AllReduce", ADD, replica_groups=REPLICA_GROUPS,
                    ins=[b_in[1][:].opt()], outs=[b_out[1][:].opt()])
            residual_add(b_out, "mlp_res", final=True)
            mpool_ctx.__exit__(None, None, None)

    nc.compile()
    return nc


# ---------------------------------------------------------------- entry
def _mask_sig(cls, pat):
    return (tuple(tuple(row) for row in cls), pat.tobytes())


def kernel(**inputs) -> np.ndarray:
    shared, per_rank, sa_cls, ca_cls = _prep_host(inputs)
    nb_sa, nb_ca = shared['mask_sa'].shape[0], shared['mask_ca'].shape[0]

    key = (_mask_sig(sa_cls, shared['mask_sa']),
           _mask_sig(ca_cls, shared['mask_ca']))
    if key not in _CACHE:
        _CACHE[key] = _build(sa_cls, ca_cls, nb_sa, nb_ca)
    nc = _CACHE[key]

    x = np.asarray(inputs['x'], dtype=np.float32)
    xb = [np.ascontiguousarray(x[g]).astype(BF_NP) for g in range(B)]
    in_maps = []
    for core in range(NCORES):
        g, r = core // TPG, core % TPG
        m = dict(shared)
        m['x'] = xb[g]
        m.update(per_rank[r])
        in_maps.append(m)

    res = run_bass_kernel_spmd(nc, in_maps, core_ids=list(range(NCORES)))
    out = np.stack([res.results[0]['out'], res.results[TPG]['out']], axis=0)
    return out.astype(np.float32)


# revision 20
# speedup vs baseline: 1.2642x; 1.1326x over previous
"""Self-contained Trainium2 Bass kernel for nn_ANEDecoderLayer (ANE decoder layer).

Shapes (hardcoded): B=2, C=2048, S=1024, H=16, HD=128, FF=8192, fp32 I/O.

Sharding: hybrid batch(2) x tensor-parallel(4) over 8 NeuronCores.
  core = g*4 + r:  g = batch index, r = TP rank.
  Within each group of 4 cores: heads sharded 4/core, d_ff sharded 2048/core.
  Block outputs (row-split Wo / W_down partial sums) are AllReduced in bf16
  within each 4-core group: replica_groups [[0,1,2,3],[4,5,6,7]].

Host-side preprocessing:
  - RMSNorm weights folded into the following matmul weights.
  - Weights pre-transposed, pre-tiled for lhsT layout, cast to bf16.
  - sin_k/cos_k pre-scaled by 1/sqrt(HD) (folds attention scale into K).
  - kv cache scatter (kv_write_idx) folded into a row-permutation of the mask.
  - additive masks converted to multiplicative exp(mask) tile patterns with
    per-tile classification (all-ones -> no op, all-zero -> tile skipped,
    else multiply by a deduplicated pattern tile).

Compute: matmuls in bf16 (fp32 PSUM accumulation), RMSNorm statistics in
fp32, softmax without max-subtraction (scores are O(5); exp(-1e9)=0 handled
by tile skipping), residual stream held in fp32 in SBUF.
"""

import numpy as np
import ml_dtypes

import concourse.mybir as mybir
import concourse.tile as tile
from concourse import bacc
from concourse.bass_utils import run_bass_kernel_spmd

# ---------------------------------------------------------------- constants
B, C, S, H, HD, FF = 2, 2048, 1024, 16, 128, 8192
EPS = 1e-5
SCALE = 1.0 / float(np.sqrt(HD))

NCORES = 8
TPG = 4                      # tensor-parallel group size
HPC = H // TPG               # heads per core = 4
OCA = HPC * HD               # attention out-channels per core = 512
FFC = FF // TPG              # ff channels per core = 2048

CT = C // 128                # 16 c-chunks
ST = S // 512                # 2 s-chunks of 512
KT = S // 128                # 8 k-chunks
FFT = FFC // 128             # 16 ff-chunks per core

F32 = mybir.dt.float32
BF = mybir.dt.bfloat16
AF = mybir.ActivationFunctionType
MULT = mybir.AluOpType.mult
ADD = mybir.AluOpType.add
BF_NP = ml_dtypes.bfloat16

REPLICA_GROUPS = [[0, 1, 2, 3], [4, 5, 6, 7]]

_CACHE: dict = {}


# ---------------------------------------------------------------- host prep
def _pack_lhsT(wT: np.ndarray) -> np.ndarray:
    """wT: (K, M) contraction-major weight. Returns (M//128, 128, K) bf16 where
    pack[m][p, kc*128+f] = wT[kc*128+p, m*128+f]; a DMA of pack[m] gives an
    SBUF tile whose slice [:, kc*128:(kc+1)*128] is the lhsT for contraction
    chunk kc -> output chunk m."""
    K, M = wT.shape
    Kt, Mt = K // 128, M // 128
    t = wT.reshape(Kt, 128, Mt, 128)              # [kc, p, m, f]
    t = t.transpose(2, 1, 0, 3).reshape(Mt, 128, K)
    return np.ascontiguousarray(t.astype(BF_NP))


def _classify_mask(mask_eff: np.ndarray):
    """mask_eff: (S, S) additive mask, (k, q) orientation. Returns
    (cls, patterns): cls[qc][kc] in {'c' (clean), 's' (skip), int idx};
    patterns: (NB, 128, 512) bf16 multiplicative tiles."""
    mm = np.exp(np.minimum(mask_eff.astype(np.float64), 0.0)).astype(np.float32)
    # positive masks would overflow exp; reference masks are <= 0
    if mask_eff.max() > 0:
        mm = np.exp(mask_eff.astype(np.float64)).astype(np.float32)
    patterns = []
    keys = {}
    cls = [[None] * KT for _ in range(ST)]
    for qc in range(ST):
        for kc in range(KT):
            sub = mm[kc * 128:(kc + 1) * 128, qc * 512:(qc + 1) * 512]
            if np.all(sub == 1.0):
                cls[qc][kc] = 'c'
            elif np.all(sub == 0.0):
                cls[qc][kc] = 's'
            else:
                kb = sub.tobytes()
                if kb not in keys:
                    keys[kb] = len(patterns)
                    patterns.append(sub.astype(BF_NP))
                cls[qc][kc] = keys[kb]
    if patterns:
        pat = np.stack(patterns)
    else:
        pat = np.zeros((1, 128, 512), BF_NP)
    return cls, pat


def _prep_host(inputs):
    """Returns (shared_map, per_rank_maps, sa_cls, ca_cls)."""
    g = lambda k: np.asarray(inputs[k], dtype=np.float32)

    sinq = np.ascontiguousarray(g('sin_q').reshape(HD, S))
    cosq = np.ascontiguousarray(g('cos_q').reshape(HD, S))
    sink = np.ascontiguousarray(g('sin_k').reshape(HD, S) * SCALE)
    cosk = np.ascontiguousarray(g('cos_k').reshape(HD, S) * SCALE)

    idx = np.asarray(inputs['kv_write_idx']).astype(np.int64)
    if not np.array_equal(np.sort(idx), np.arange(S)):
        raise NotImplementedError("kv_write_idx must be a permutation of arange(S)")
    sa_mask = g('self_attn_mask').reshape(S, S)[idx, :]     # effective (k, q) mask
    ca_mask = g('cross_attn_mask').reshape(S, S)
    sa_cls, sa_pat = _classify_mask(sa_mask)
    ca_cls, ca_pat = _classify_mask(ca_mask)

    P_rot = np.zeros((HD, HD), np.float32)
    P_rot[np.arange(64), np.arange(64, 128)] = -1.0
    P_rot[np.arange(64, 128), np.arange(64)] = 1.0

    shared = {
        'sinq': sinq.astype(BF_NP), 'cosq': cosq.astype(BF_NP),
        'sink': sink.astype(BF_NP), 'cosk': cosk.astype(BF_NP),
        'ones_col': np.ones((128, 1), np.float32),
        'ones_col_bf': np.ones((128, 1), BF_NP),
        'ones_row': np.ones((1, 128), np.float32),
        'ident': np.eye(128).astype(BF_NP),
        'protT': np.ascontiguousarray(P_rot.T).astype(BF_NP),
        'mask_sa': sa_pat, 'mask_ca': ca_pat,
    }

    w_sa, w_ca, w_mlp = g('w_sa'), g('w_ca'), g('w_mlp')
    per_rank = []
    for r in range(TPG):
        asl = slice(r * OCA, (r + 1) * OCA)
        fsl = slice(r * FFC, (r + 1) * FFC)
        m = {}
        for tag, wn, on in (('sa', 'w_sa', None), ('ca', 'w_ca', None)):
            wnorm = w_sa if tag == 'sa' else w_ca
            for p in ('q', 'k', 'v'):
                W = g(f'w{p}_{tag}')[asl, :] * wnorm[None, :]
                m[f'w{p}_{tag}'] = _pack_lhsT(np.ascontiguousarray(W.T))
            Wo = g(f'wo_{tag}')[:, asl]
            m[f'wo_{tag}'] = _pack_lhsT(np.ascontiguousarray(Wo.T))
        for p, key in (('g', 'w_gate'), ('u', 'w_up')):
            W = g(key)[fsl, :] * w_mlp[None, :]
            m[f'w{p}'] = _pack_lhsT(np.ascontiguousarray(W.T))
        Wd = g('w_down')[:, fsl]
        m['wd'] = _pack_lhsT(np.ascontiguousarray(Wd.T))
        per_rank.append(m)

    return shared, per_rank, sa_cls, ca_cls


# ---------------------------------------------------------------- builder
def _build(sa_cls, ca_cls, nb_sa, nb_ca):
    nc = bacc.Bacc("TRN2", target_bir_lowering=False, debug=False,
                   num_devices=NCORES)

    d_x = nc.declare_dram_parameter("x", [C, S], BF, isOutput=False)
    d_tab = {k: nc.declare_dram_parameter(k, [HD, S], BF, isOutput=False)
             for k in ('sinq', 'cosq', 'sink', 'cosk')}
    d_oc = nc.declare_dram_parameter("ones_col", [128, 1], F32, isOutput=False)
    d_ocb = nc.declare_dram_parameter("ones_col_bf", [128, 1], BF, isOutput=False)
    d_or = nc.declare_dram_parameter("ones_row", [1, 128], F32, isOutput=False)
    d_id = nc.declare_dram_parameter("ident", [128, 128], BF, isOutput=False)
    d_pr = nc.declare_dram_parameter("protT", [128, 128], BF, isOutput=False)
    d_msa = nc.declare_dram_parameter("mask_sa", [nb_sa, 128, 512], BF, isOutput=False)
    d_mca = nc.declare_dram_parameter("mask_ca", [nb_ca, 128, 512], BF, isOutput=False)
    d_w = {}
    for t in ('sa', 'ca'):
        for p in ('q', 'k', 'v'):
            d_w[f'w{p}_{t}'] = nc.declare_dram_parameter(
                f'w{p}_{t}', [OCA // 128, 128, C], BF, isOutput=False)
        d_w[f'wo_{t}'] = nc.declare_dram_parameter(
            f'wo_{t}', [CT, 128, OCA], BF, isOutput=False)
    for k in ('wg', 'wu', 'wd'):
        kdim = C if k != 'wd' else FFC
        d_w[k] = nc.declare_dram_parameter(k, [FFT, 128, kdim], BF, isOutput=False)
    d_out = nc.declare_dram_parameter("out", [C, S], F32, isOutput=True)

    with tile.TileContext(nc) as tc:
        with (
            tc.tile_pool(name="const", bufs=1) as cpool,
            tc.tile_pool(name="xp", bufs=1) as xpool,
            tc.tile_pool(name="hp", bufs=1) as hpool,
            tc.tile_pool(name="wb", bufs=3) as wpool,
            tc.tile_pool(name="oo", bufs=2) as opool,
            tc.tile_pool(name="sm", bufs=2) as spool,
            tc.tile_pool(name="dram", bufs=1, space="DRAM") as dpool,
            tc.tile_pool(name="psA", bufs=5, space="PSUM") as psA,
            tc.tile_pool(name="psS", bufs=1, space="PSUM") as psS,
            tc.tile_pool(name="psT", bufs=1, space="PSUM") as psT,
        ):
            # ---------------- constants / tables ----------------
            def ptile(pool, shape, dt, name):
                return pool.tile(shape, dt, name=name, tag=name)

            ones_col = ptile(cpool, [128, 1], F32, "ones_col")
            ones_col_bf = ptile(cpool, [128, 1], BF, "ones_col_bf")
            ones_row = ptile(cpool, [1, 128], F32, "ones_row")
            ident = ptile(cpool, [128, 128], BF, "ident")
            protT = ptile(cpool, [128, 128], BF, "protT")
            eps_t = ptile(cpool, [128, 1], F32, "eps_t")
            nc.sync.dma_start(ones_col[:], d_oc.ap())
            nc.sync.dma_start(ones_col_bf[:], d_ocb.ap())
            nc.sync.dma_start(ones_row[:], d_or.ap())
            nc.sync.dma_start(ident[:], d_id.ap())
            nc.sync.dma_start(protT[:], d_pr.ap())
            nc.vector.memset(eps_t[:], EPS)
            tabs = {}
            for k in d_tab:
                tabs[k] = ptile(cpool, [HD, S], BF, f"tab_{k}")
                nc.sync.dma_start(tabs[k][:], d_tab[k].ap())
            used_sa = {c for row in sa_cls for c in row if isinstance(c, int)}
            used_ca = {c for row in ca_cls for c in row if isinstance(c, int)}
            msk_sa, msk_ca = {}, {}
            for i in sorted(used_sa):
                msk_sa[i] = ptile(cpool, [128, 512], BF, f"msa{i}")
                nc.sync.dma_start(msk_sa[i][:], d_msa.ap()[i])
            for i in sorted(used_ca):
                msk_ca[i] = ptile(cpool, [128, 512], BF, f"mca{i}")
                nc.sync.dma_start(msk_ca[i][:], d_mca.ap()[i])

            # ---------------- residual stream x ----------------
            xt = [ptile(xpool, [128, S], BF, f"x{cc}") for cc in range(CT)]
            for cc in range(CT):
                nc.sync.dma_start(xt[cc][:], d_x.ap()[cc * 128:(cc + 1) * 128, :])
            ht = [ptile(hpool, [128, S], BF, f"h{cc}") for cc in range(CT)]

            # ---------------- helpers ----------------
            def rmsnorm(scope):
                """ht[:] = xt * rsqrt(mean_c(xt^2) + eps), bf16."""
                with nc.named_scope(scope):
                    for sc in range(ST):
                        ss = psS.tile([1, 512], F32, tag="nsum")
                        for cc in range(CT):
                            sq = spool.tile([128, 512], F32, tag="sq")
                            nc.scalar.activation(
                                sq[:], xt[cc][:, sc * 512:(sc + 1) * 512], AF.Square)
                            nc.tensor.matmul(ss[:], ones_col[:], sq[:],
                                             start=(cc == 0), stop=(cc == CT - 1))
                        rs = spool.tile([1, 512], F32, tag="rs")
                        nc.scalar.activation(rs[:], ss[:], AF.Sqrt,
                                             bias=eps_t[:1, :], scale=1.0 / C)
                        rr = spool.tile([1, 512], F32, tag="rr")
                        nc.vector.reciprocal_approx_fast(rr[:], rs[:])
                        bc = psA.tile([128, 512], F32, tag="acc")
                        nc.tensor.matmul(bc[:], ones_row[:], rr[:],
                                         start=True, stop=True)
                        for cc in range(CT):
                            nc.vector.tensor_tensor(
                                ht[cc][:, sc * 512:(sc + 1) * 512],
                                xt[cc][:, sc * 512:(sc + 1) * 512],
                                bc[:], op=MULT)

            def residual_add(b_out, scope, final=False):
                """xt += AR result (bf16 in dram halves); if final, write out."""
                with nc.named_scope(scope):
                    for cc in range(CT):
                        h, row = cc // (CT // 2), (cc % (CT // 2)) * 128
                        ar = opool.tile([128, S], BF, tag="ar")
                        nc.sync.dma_start(ar[:], b_out[h][row:row + 128, :])
                        if final:
                            ot = opool.tile([128, S], F32, tag="obuf")
                            nc.vector.tensor_tensor(ot[:], xt[cc][:], ar[:], op=ADD)
                            nc.sync.dma_start(
                                d_out.ap()[cc * 128:(cc + 1) * 128, :], ot[:])
                        else:
                            nc.vector.tensor_tensor(xt[cc][:], xt[cc][:], ar[:], op=ADD)

            def attention(t, cls, msk, apool):
                """One attention block (t='sa'|'ca'). Returns bounce-out dram tile."""
                rmsnorm(f"{t}_norm")
                qk_rope = {}
                vT = [apool.tile([128, S], BF, name=f"vT{t}{oc}", tag=f"vT{oc}",
                                 bufs=1) for oc in range(HPC)]
                with nc.named_scope(f"{t}_qkv"):
                    for p in ('q', 'k', 'v'):
                        for oc in range(HPC):
                            if p != 'v':
                                dst = apool.tile([128, S], BF,
                                                 name=f"{p}r{t}{oc}",
                                                 tag=f"{p}r{oc}", bufs=1)
                                qk_rope[(p, oc)] = dst
                            wsb = wpool.tile([128, C], BF, tag="wbig")
                            nc.sync.dma_start(wsb[:], d_w[f'w{p}_{t}'].ap()[oc])
                            pss = [psA.tile([128, 512], F32, tag="acc",
                                            name=f"ps{p}{oc}{j}") for j in range(ST)]
                            for cc in range(CT):
                                wsl = wsb[:, cc * 128:(cc + 1) * 128]
                                for sc in range(ST):
                                    nc.tensor.matmul(
                                        pss[sc][:],
                                        wsl, ht[cc][:, sc * 512:(sc + 1) * 512],
                                        start=(cc == 0), stop=(cc == CT - 1))
                            for sc in range(ST):
                                s0 = slice(sc * 512, (sc + 1) * 512)
                                ps = pss[sc]
                                if p == 'v':
                                    vsb = spool.tile([128, 512], BF, tag="lin")
                                    nc.scalar.activation(vsb[:], ps[:], AF.Copy)
                                    for j in range(4):
                                        kc = sc * 4 + j
                                        tp = psT.tile([128, 128], BF, tag="tr")
                                        nc.tensor.transpose(
                                            tp[:], vsb[:, j * 128:(j + 1) * 128],
                                            ident[:])
                                        nc.scalar.activation(
                                            vT[oc][:, kc * 128:(kc + 1) * 128],
                                            tp[:], AF.Copy)
                                else:
                                    lin = spool.tile([128, 512], BF, tag="lin")
                                    nc.scalar.activation(lin[:], ps[:], AF.Copy)
                                    rot = psA.tile([128, 512], F32, tag="acc")
                                    nc.tensor.matmul(rot[:], protT[:], lin[:],
                                                     start=True, stop=True)
                                    sin = tabs['sinq' if p == 'q' else 'sink']
                                    cos = tabs['cosq' if p == 'q' else 'cosk']
                                    dst = qk_rope[(p, oc)]
                                    nc.vector.tensor_tensor(
                                        dst[:, s0], lin[:], cos[:, s0], op=MULT)
                                    s2 = spool.tile([128, 512], BF, tag="rsc")
                                    nc.vector.tensor_tensor(
                                        s2[:], rot[:], sin[:, s0], op=MULT)
                                    nc.vector.tensor_tensor(
                                        dst[:, s0], dst[:, s0], s2[:], op=ADD)
                att = [apool.tile([128, S], BF, name=f"att{t}{oc}",
                                  tag=f"att{oc}", bufs=1) for oc in range(HPC)]
                with nc.named_scope(f"{t}_attn"):
                    for qc in range(ST):
                        for oc in range(HPC):
                            qr, kr = qk_rope[('q', oc)], qk_rope[('k', oc)]
                            s0 = slice(qc * 512, (qc + 1) * 512)
                            valid = [kc for kc in range(KT) if cls[qc][kc] != 's']
                            probs = {}
                            for kc in valid:
                                sp = psA.tile([128, 512], F32, tag="acc")
                                nc.tensor.matmul(
                                    sp[:], kr[:, kc * 128:(kc + 1) * 128],
                                    qr[:, s0], start=True, stop=True)
                                pt = apool.tile([128, 512], BF, tag="probs",
                                                bufs=12)
                                nc.scalar.activation(pt[:], sp[:], AF.Exp)
                                if cls[qc][kc] != 'c':
                                    nc.vector.tensor_tensor(
                                        pt[:], pt[:], msk[cls[qc][kc]][:], op=MULT)
                                probs[kc] = pt
                            dn = psS.tile([1, 512], F32, tag="dsum")
                            for i, kc in enumerate(valid):
                                nc.tensor.matmul(dn[:], ones_col_bf[:],
                                                 probs[kc][:],
                                                 start=(i == 0),
                                                 stop=(i == len(valid) - 1))
                            ra = spool.tile([1, 512], F32, tag="ra")
                            nc.vector.reciprocal_approx_fast(ra[:], dn[:])
                            rb = psA.tile([128, 512], F32, tag="acc")
                            nc.tensor.matmul(rb[:], ones_row[:], ra[:],
                                             start=True, stop=True)
                            # DVE reads only one PSUM input; stage bcast in SBUF
                            rbs = spool.tile([128, 512], F32, tag="rbs")
                            nc.scalar.activation(rbs[:], rb[:], AF.Copy)
                            pa = psA.tile([128, 512], F32, tag="acc")
                            for i, kc in enumerate(valid):
                                nc.tensor.matmul(
                                    pa[:], vT[oc][:, kc * 128:(kc + 1) * 128],
                                    probs[kc][:],
                                    start=(i == 0), stop=(i == len(valid) - 1))
                            nc.vector.tensor_tensor(att[oc][:, s0], pa[:], rbs[:],
                                                    op=MULT)
                b_in = [dpool.tile([C // 2, S], BF, name=f"bin_{t}{h}",
                                   tag=f"bin_{t}{h}") for h in range(2)]
                b_out = [dpool.tile([C // 2, S], BF, name=f"bout_{t}{h}",
                                    tag=f"bout_{t}{h}") for h in range(2)]
                with nc.named_scope(f"{t}_wo"):
                    for cc in range(CT):
                        wsb = wpool.tile([128, OCA], BF, tag="wsm")
                        nc.sync.dma_start(wsb[:], d_w[f'wo_{t}'].ap()[cc])
                        osb = opool.tile([128, S], BF, tag="obuf")
                        pss = [psA.tile([128, 512], F32, tag="acc",
                                        name=f"pso{cc}{j}") for j in range(ST)]
                        for ac in range(HPC):
                            wsl = wsb[:, ac * 128:(ac + 1) * 128]
                            for sc in range(ST):
                                nc.tensor.matmul(
                                    pss[sc][:], wsl,
                                    att[ac][:, sc * 512:(sc + 1) * 512],
                                    start=(ac == 0), stop=(ac == HPC - 1))
                        for sc in range(ST):
                            s0 = slice(sc * 512, (sc + 1) * 512)
                            nc.scalar.activation(osb[:, s0], pss[sc][:], AF.Copy)
                        h, row = cc // (CT // 2), (cc % (CT // 2)) * 128
                        nc.sync.dma_start(b_in[h][row:row + 128, :], osb[:])
                        if cc == CT // 2 - 1:
                            nc.gpsimd.collective_compute(
                                "AllReduce", ADD, replica_groups=REPLICA_GROUPS,
                                ins=[b_in[0][:].opt()], outs=[b_out[0][:].opt()])
                    nc.gpsimd.collective_compute(
                        "AllReduce", ADD, replica_groups=REPLICA_GROUPS,
                        ins=[b_in[1][:].opt()], outs=[b_out[1][:].opt()])
                return b_out

            # ================= self-attention =================
            with tc.tile_pool(name="ap", bufs=1) as apool:
                b = attention('sa', sa_cls, msk_sa, apool)
                residual_add(b, "sa_res")
                # ============= cross-attention =============
                b = attention('ca', ca_cls, msk_ca, apool)
                residual_add(b, "ca_res")
            # ================= MLP =================
            rmsnorm("mlp_norm")
            mpool_ctx = tc.tile_pool(name="mp", bufs=1)
            mpool = mpool_ctx.__enter__()
            gact = [mpool.tile([128, S], BF, name=f"gact{f}", tag=f"gact{f}",
                               bufs=1) for f in range(FFT)]
            with nc.named_scope("mlp_up"):
                for f in range(FFT):
                    wg = wpool.tile([128, C], BF, tag="wbig")
                    nc.sync.dma_start(wg[:], d_w['wg'].ap()[f])
                    wu = wpool.tile([128, C], BF, tag="wbig")
                    nc.sync.dma_start(wu[:], d_w['wu'].ap()[f])
                    pgs = [psA.tile([128, 512], F32, tag="acc", name=f"pg{f}{j}")
                           for j in range(ST)]
                    pus = [psA.tile([128, 512], F32, tag="acc", name=f"pu{f}{j}")
                           for j in range(ST)]
                    for cc in range(CT):
                        wsl = wg[:, cc * 128:(cc + 1) * 128]
                        for sc in range(ST):
                            nc.tensor.matmul(pgs[sc][:], wsl,
                                             ht[cc][:, sc * 512:(sc + 1) * 512],
                                             start=(cc == 0), stop=(cc == CT - 1))
                    for cc in range(CT):
                        wsl = wu[:, cc * 128:(cc + 1) * 128]
                        for sc in range(ST):
                            nc.tensor.matmul(pus[sc][:], wsl,
                                             ht[cc][:, sc * 512:(sc + 1) * 512],
                                             start=(cc == 0), stop=(cc == CT - 1))
                    for sc in range(ST):
                        s0 = slice(sc * 512, (sc + 1) * 512)
                        gs = spool.tile([128, 512], BF, tag="lin")
                        nc.scalar.activation(gs[:], pgs[sc][:], AF.Silu)
                        nc.vector.tensor_tensor(gact[f][:, s0], gs[:], pus[sc][:],
                                                op=MULT)
            b_in = [dpool.tile([C // 2, S], BF, name=f"bin_mlp{h}",
                               tag=f"bin_mlp{h}") for h in range(2)]
            b_out = [dpool.tile([C // 2, S], BF, name=f"bout_mlp{h}",
                                tag=f"bout_mlp{h}") for h in range(2)]
            with nc.named_scope("mlp_down"):
                for cc in range(CT):
                    wd = wpool.tile([128, FFC], BF, tag="wbig")
                    nc.sync.dma_start(wd[:], d_w['wd'].ap()[cc])
                    osb = opool.tile([128, S], BF, tag="obuf")
                    pss = [psA.tile([128, 512], F32, tag="acc", name=f"pd{cc}{j}")
                           for j in range(ST)]
                    for f in range(FFT):
                        wsl = wd[:, f * 128:(f + 1) * 128]
                        for sc in range(ST):
                            nc.tensor.matmul(pss[sc][:], wsl,
                                             gact[f][:, sc * 512:(sc + 1) * 512],
                                             start=(f == 0), stop=(f == FFT - 1))
                    for sc in range(ST):
                        s0 = slice(sc * 512, (sc + 1) * 512)
                        nc.scalar.activation(osb[:, s0], pss[sc][:], AF.Copy)
                    h, row = cc // (CT // 2), (cc % (CT // 2)) * 128
                    nc.sync.dma_start(b_in[h][row:row + 128, :], osb[:])
                    if cc == CT // 2 - 1:
                        nc.gpsimd.collective_compute(
                            "AllReduce", ADD, replica_groups=REPLICA_GROUPS,
                            ins=[b_in[0][:].opt()], outs=[b_out[0][:].opt()])
                nc.gpsimd.collective_compute(
                    "AllReduce", ADD, replica_groups=REPLICA_GROUPS,
                    ins=[b_in[1][:].opt()], outs=[b_out[1][:].opt()])
            residual_add(b_out, "mlp_res", final=True)
            mpool_ctx.__exit__(None, None, None)

    nc.compile()
    return nc


# ---------------------------------------------------------------- entry
def _mask_sig(cls, pat):
    return (tuple(tuple(row) for row in cls), pat.tobytes())


def kernel(**inputs) -> np.ndarray:
    shared, per_rank, sa_cls, ca_cls = _prep_host(inputs)
    nb_sa, nb_ca = shared['mask_sa'].shape[0], shared['mask_ca'].shape[0]

    key = (_mask_sig(sa_cls, shared['mask_sa']),
           _mask_sig(ca_cls, shared['mask_ca']))
    if key not in _CACHE:
        _CACHE[key] = _build(sa_cls, ca_cls, nb_sa, nb_ca)
    nc = _CACHE[key]

    x = np.asarray(inputs['x'], dtype=np.float32)
    xb = [np.ascontiguousarray(x[g]).astype(BF_NP) for g in range(B)]
    in_maps = []
    for core in range(NCORES):
        g, r = core // TPG, core % TPG
        m = dict(shared)
        m['x'] = xb[g]
        m.update(per_rank[r])
        in_maps.append(m)

    res = run_bass_kernel_spmd(nc, in_maps, core_ids=list(range(NCORES)))
    out = np.stack([res.results[0]['out'], res.results[TPG]['out']], axis=0)
    return out.astype(np.float32)
